# revision 1
# baseline (speedup 1.0000x reference)
"""Trainium2 Bass kernel for nn_Attention_7602092114471.

Full multi-head attention block:
  qkv = x @ w_qkv.T ; split q,k,v into 12 heads of d=64
  q = rope(q * d**-0.5) ; k = rope(k)   (lucidrains interleaved RoPE)
  attn = softmax(q @ k.T) ; out = (attn @ v) reassembled, @ w_proj.T + b_proj

Shapes: x [2, 2048, 768], w_qkv [2304, 768], w_proj [768, 768], b_proj [768].

Sharding: 24 (batch, head) pairs -> 8 cores x 3 heads. Core c handles batch
c//4, heads {3g, 3g+1, 3g+2} with g = c%4. Each core computes its heads'
q/k/v projections, attention, and a partial output projection over its
3 heads' feature columns. The host sums the 4 partial projections per batch
(the tensor-parallel all-reduce, done on host during unshard) and adds bias.

Layout (hardcoded for these shapes):
  * x passed transposed per batch (xT [768, 2048], contraction on
    partitions), DMA'd in per-strip chunks so the first projection chain
    starts a few us in instead of after the full 6.3MB load.
  * q/k produced FEATURE-major ([d, token]); scores computed transposed,
    ST[j, i] = k_j . q_i, so the PV matmul (contraction over j) needs no
    transposes anywhere.
  * Softmax: exp on ScalarE straight out of PSUM (constant -8 bias, cancels
    in normalization); the denominator L_i comes for free from the PV matmul
    via 64 ones-columns in the stationary operand (PV output rows 64..127).
    Normalization: both heads' numerator/denominator are copied out of PSUM
    packed into [128, 512] tiles (fast bank release), one DVE reciprocal
    covers two heads, then per-head multiplies. (The ~5x-faster custom-DVE
    reciprocal_approx_fast miscomputes on this runtime - stock ops only.)
  * RoPE: interleaved rotation conjugated into rotate-half-by-32 by
    permuting the q/k weight rows on the host; the swapped-partner
    projection comes from extra (permuted) weight columns, so the rotation
    is 3 DVE ops per [128, 512] tile.
  * v reaches token-major via PE transposes staged through the projection
    PSUM bank (idle during strip 0), scattered by DVE.
  * Dtypes: x / qkv weights / scores fp32r; post-rope q/k, v, exp(scores),
    P, proj weights and output partials bf16 (error budget 2e-2, measured
    4.2e-3).

Single software pipeline (the core of the 314us -> ~216us speedup):
ScalarE exp is the irreducible pacer (12.6M exps/core at 1 elem/lane/cycle
@ 1.2GHz = ~25us per 512-query i-strip). After a ~10us mini-prologue
(strip 0's own k/q2k2/q01 chains), the score/exp stream starts and ALL
remaining work is issued as interleaved PE filler between exp groups so
the PE stays dense (HAM stays at K=8/8) and ScalarE almost never waits:
  * strip 0's fillers: the k/q2k2/q01 projection chains for strips 1-3
    (+ their RoPE on DVE), the v projections and transposes;
  * strip s>=1 fillers: strip s-1's PV chains (heads sequential through a
    2-bank PSUM rotation), its normalization, and the projection of
    strip s-2 (s-1 for the last strip) in a dedicated bank.
Scores for key-strip t only need k of strip t, which is why the exp stream
can start at ~10us. Per key block: one N=1024 exp ACT for h0+h1 (4 PSUM
banks, double-buffered so neither engine waits) and one N=512 ACT for h2
(1 bank, trailing one block). PSUM: 4 + 1 + 2 (chains/PV) + 1 (proj) = 8.
"""

import numpy as np

import concourse.bass as bass
import concourse.mybir as mybir
import concourse.tile as tile
from concourse import bacc, bass_utils
from concourse.masks import make_identity

# Problem constants (hardcoded per contract; kernel.py must be self-contained).
B = 2
N = 2048
C = 768
H = 12
D = 64
ROPE_THETA = 10000.0
NCORES = 8
HPC = 3  # heads per core

F32 = mybir.dt.float32
BF16 = mybir.dt.bfloat16

import os
_BF = lambda name, dflt="1": (BF16 if os.environ.get(name, dflt) == "1" else mybir.dt.float32r)
QK_DT = _BF("K_BF_QK")    # q01/k01/q2d/k2d (score matmuls)
V_DT = _BF("K_BF_V")      # v_sb + e (PV matmul)
P_DT = _BF("K_BF_P")      # P0/P1 + wp (proj matmul)
O_DT = _BF("K_BF_O")      # outT partials
DEBUG_DUMP = os.environ.get("K_DEBUG_DUMP", "0") == "1"

MM_DT = "float32r"

IS = 512                  # strip width for phase-1 projections
NSTRIP = N // IS          # 4
ISA = 512                 # attention i-strip width
NSA = N // ISA
EXPG = 2                  # jb per score group
NJB = N // 128            # 16 key blocks
NGRP = NJB // EXPG        # 8 score groups per strip
KT = C // 128             # 6 contraction tiles for the projections
EXP_BIAS = -8.0           # constant shift inside exp; cancels in normalization


def _mmdt():
    return mybir.dt.float32r if MM_DT == "float32r" else F32


def build_nc():
    """Build the per-core Bass module (same NEFF runs SPMD on all 8 cores)."""
    nc = bacc.Bacc(
        "TRN2",
        target_bir_lowering=False,
        debug=False,
        enable_asserts=False,
    )

    mmdt = _mmdt()
    xT = nc.dram_tensor("xT", [C, N], mmdt, kind="ExternalInput").ap()
    w_feat = nc.dram_tensor("w_feat", [C, 15 * D], mmdt, kind="ExternalInput").ap()
    wp = nc.dram_tensor("wp", [256, C], P_DT, kind="ExternalInput").ap()
    cosT = nc.dram_tensor("cosT", [128, N], F32, kind="ExternalInput").ap()
    sinT = nc.dram_tensor("sinT", [128, N], F32, kind="ExternalInput").ap()
    ones = nc.dram_tensor("ones", [128, D], V_DT, kind="ExternalInput").ap()
    outT = nc.dram_tensor("outT", [C, N], O_DT, kind="ExternalOutput").ap()
    dbg = None
    if DEBUG_DUMP:
        dbg = {
            nm: nc.dram_tensor(f"dbg_{nm}", shp, dt, kind="ExternalOutput").ap()
            for nm, shp, dt in [
                ("q01", [128, N], QK_DT), ("k01", [128, N], QK_DT),
                ("q2d", [128, N], QK_DT), ("k2d", [128, N], QK_DT),
                ("v_sb", [128, NJB * 384], V_DT),
                ("e0", [128, NJB * ISA], V_DT), ("e1", [128, NJB * ISA], V_DT),
                ("e2", [128, NJB * ISA], V_DT),
                ("P0", [128, N], P_DT), ("P1", [128, N], P_DT),
            ]
        }

    with tile.TileContext(nc) as tc:
        _kernel_body(tc, nc, xT, w_feat, wp, cosT, sinT, ones, outT, dbg)
    nc.compile()
    return nc


def _rope_tile(nc, pool, dst, psrc, psrc_s, cos_sb, sin_sb, s):
    """RoPE on one PSUM tile strip: dst = psrc*cos + psrc_s*sinmod (bf16 out,
    both products formed in fp32, one rounding)."""
    rows = psrc.shape[0]
    ss = slice(s * IS, (s + 1) * IS)
    tmp1 = pool.tile([128, IS], F32, name="rope_tmp1", tag="rope_tmp1")
    tmp2 = pool.tile([128, IS], F32, name="rope_tmp2", tag="rope_tmp2")
    nc.vector.tensor_mul(out=tmp1[:rows, :], in0=psrc, in1=cos_sb[:rows, ss])
    nc.vector.tensor_mul(out=tmp2[:rows, :], in0=psrc_s, in1=sin_sb[:rows, ss])
    nc.vector.tensor_add(out=dst[:rows, ss], in0=tmp1[:rows, :], in1=tmp2[:rows, :])


def _kernel_body(tc, nc, xT, w_feat, wp, cosT, sinT, ones, outT, dbg=None):
    import contextlib

    ctx = contextlib.ExitStack()
    with ctx:
        persist = ctx.enter_context(tc.tile_pool(name="persist", bufs=1))
        rope_pool = ctx.enter_context(tc.tile_pool(name="rope", bufs=4))
        attnA = ctx.enter_context(tc.tile_pool(name="attnA", bufs=1))
        # attention-phase PSUM (whole kernel): 4 (h0h1 scores, double-
        # buffered) + 1 (h2) + 2 (chains/PV rotation) + 1 (proj/transposes)
        sts01p = ctx.enter_context(tc.tile_pool(name="sts01", bufs=2, space="PSUM"))
        sts2p = ctx.enter_context(tc.tile_pool(name="sts2", bufs=1, space="PSUM"))
        wkps = ctx.enter_context(tc.tile_pool(name="wkps", bufs=2, space="PSUM"))
        prps = ctx.enter_context(tc.tile_pool(name="prps", bufs=1, space="PSUM"))

        # ---- persistent SBUF tensors -------------------------------------
        q01 = persist.tile([128, N], QK_DT, name="q01")
        k01 = persist.tile([128, N], QK_DT, name="k01")
        q2d = persist.tile([128, N], QK_DT, name="q2d")
        k2d = persist.tile([128, N], QK_DT, name="k2d")
        v_sb = persist.tile([128, NJB, 3 * 128], V_DT, name="v_sb")
        P0 = persist.tile([128, N], P_DT, name="P0")  # heads h0 | h1
        P1 = persist.tile([128, N], P_DT, name="P1")  # h2 duplicated
        wp_sb = persist.tile([128, 2, C], P_DT, name="wp_sb")
        bias_sb = persist.tile([128, 1], F32, name="bias_sb")
        nc.vector.memset(bias_sb, EXP_BIAS)

        # strip-0 exp buffers (live for the whole kernel, allocated before
        # the big phase-1 pool so SBUF peaks stay under the limit)
        e01_0 = attnA.tile([128, NJB, 2, ISA], V_DT, name="e01_0")
        e2_0 = attnA.tile([128, NJB, ISA], V_DT, name="e2_0")

        vtpool = ctx.enter_context(tc.tile_pool(name="vtpool", bufs=1))
        ph1_stack = contextlib.ExitStack()
        ph1 = ph1_stack.enter_context(tc.tile_pool(name="ph1", bufs=1))

        w_sb = ph1.tile([128, KT, 15 * D], _mmdt(), name="w_sb")
        wr = w_feat.rearrange("(o p) f -> p o f", p=128)
        for kt in range(KT):
            nc.sync.dma_start(w_sb[:, kt], wr[:, kt])
        cos_sb = ph1.tile([128, N], F32, name="cos_sb")
        sin_sb = ph1.tile([128, N], F32, name="sin_sb")
        x_sb = [
            ph1.tile([128, N], _mmdt(), name=f"x_sb{kt}", tag=f"x_sb{kt}")
            for kt in range(KT)
        ]
        for s in range(NSTRIP):
            ss = slice(s * IS, (s + 1) * IS)
            for kt in range(KT):
                nc.sync.dma_start(x_sb[kt][:, ss], xT[kt * 128 : (kt + 1) * 128, ss])
            if s == 0:
                nc.sync.dma_start(cos_sb, cosT)
                nc.sync.dma_start(sin_sb, sinT)
        ones_dst = v_sb.rearrange("p j (h x) -> p (j h) x", x=128)[:, :, D:128]
        nc.sync.dma_start(ones_dst, ones[:, None, :].to_broadcast((128, NJB * 3, D)))
        nc.sync.dma_start(wp_sb, wp.rearrange("(o p) f -> p o f", p=128))

        ident = vtpool.tile([128, 128], F32, name="ident")
        make_identity(nc, ident)
        vT01 = vtpool.tile([128, N], F32, name="vT01")
        vT2 = vtpool.tile([64, N], F32, name="vT2")

        # w_feat column blocks (128 wide unless noted):
        #   0: q0|q1   1: swap(q0|q1)   2: k0|k1   3: swap(k0|k1)
        #   4: q2|k2   5: swap(q2|k2)   6: v0|v1   7: v2 (64 wide)
        def qkv_chain(col, m, s):
            ss = slice(s * IS, (s + 1) * IS)
            pt = wkps.tile([128, IS], F32, name="wk", tag="wk")
            for kt in range(KT):
                nc.tensor.matmul(
                    pt[:m, :],
                    w_sb[:, kt, col : col + m],
                    x_sb[kt][:, ss],
                    start=(kt == 0),
                    stop=(kt == KT - 1),
                )
            return pt

        ch_state = {}

        def k_main(s):
            ch_state["k"] = qkv_chain(2 * 128, 128, s)

        def k_swap(s):
            pt_s = qkv_chain(3 * 128, 128, s)
            _rope_tile(nc, rope_pool, k01, ch_state.pop("k"), pt_s,
                       cos_sb, sin_sb, s)

        def qk2_main(s):
            ch_state["qk2"] = qkv_chain(4 * 128, 128, s)

        def qk2_swap(s):
            pt = ch_state.pop("qk2")
            pt_s = qkv_chain(5 * 128, 128, s)
            _rope_tile(nc, rope_pool, q2d, pt[0:64, :], pt_s[0:64, :],
                       cos_sb, sin_sb, s)
            _rope_tile(nc, rope_pool, k2d, pt[64:128, :], pt_s[64:128, :],
                       cos_sb, sin_sb, s)
            # per-strip duplicate rows on ScalarE (idle while filling)
            ss = slice(s * IS, (s + 1) * IS)
            nc.scalar.copy(out=k2d[64:128, ss], in_=k2d[0:64, ss])
            nc.scalar.copy(out=q2d[64:128, ss], in_=q2d[0:64, ss])

        def q_main(s):
            ch_state["q"] = qkv_chain(0, 128, s)

        def q_swap(s):
            pt_s = qkv_chain(128, 128, s)
            _rope_tile(nc, rope_pool, q01, ch_state.pop("q"), pt_s,
                       cos_sb, sin_sb, s)

        # mini-upfront: strip 0's own projections only (~10us)
        k_main(0); k_swap(0)
        qk2_main(0); qk2_swap(0)
        q_main(0); q_swap(0)

        # ---- attention pipeline ------------------------------------------
        e01 = [e01_0, None]
        e2 = [e2_0, None]

        def score_h2(s, jb):
            e2t = e2[s % 2]
            ss = slice(s * ISA, (s + 1) * ISA)
            jbs = slice(jb * 128, (jb + 1) * 128)
            half = jb & 1
            hh = slice(half * 64, half * 64 + 64)
            st2 = sts2p.tile([128, ISA], F32, name="st2", tag="st2")
            nc.tensor.matmul(st2, k2d[hh, jbs], q2d[hh, ss], start=True, stop=True)
            nc.scalar.activation(
                out=e2t[:, jb, :], in_=st2,
                func=mybir.ActivationFunctionType.Exp, bias=bias_sb[:, :],
            )

        def score01(s, jb):
            """h0/h1 scores for one key block + one N=1024 exp ACT.
            sts01 is double-buffered so these MMs never wait on the previous
            ACT and ScalarE never waits on these MMs."""
            e01t = e01[s % 2]
            ss = slice(s * ISA, (s + 1) * ISA)
            jbs = slice(jb * 128, (jb + 1) * 128)
            st01 = sts01p.tile([128, 2, ISA], F32, name="st01", tag="st01")
            nc.tensor.matmul(st01[:, 0, :], k01[0:64, jbs], q01[0:64, ss],
                             start=True, stop=True)
            nc.tensor.matmul(st01[:, 1, :], k01[64:128, jbs], q01[64:128, ss],
                             start=True, stop=True)
            nc.scalar.activation(
                out=e01t[:, jb], in_=st01,
                func=mybir.ActivationFunctionType.Exp, bias=bias_sb[:, :],
            )

        def pv_mms(s, h, pv, jbs):
            e01t, e2t = e01[s % 2], e2[s % 2]
            for jb in jbs:
                mv = e01t[:, jb, h, :] if h < 2 else e2t[:, jb, :]
                nc.tensor.matmul(
                    pv, v_sb[:, jb, h * 128 : (h + 1) * 128], mv,
                    start=(jb == 0), stop=(jb == NJB - 1),
                )

        # ---- strip 0: scores/exp with the rest of phase 1 as filler ------
        def v_chain01(sx):
            ss = slice(sx * IS, (sx + 1) * IS)
            pt = qkv_chain(6 * 128, 128, sx)
            nc.vector.tensor_copy(out=vT01[:, ss], in_=pt[:, :])

        def v_chain2(sx):
            ss = slice(sx * IS, (sx + 1) * IS)
            pt = qkv_chain(7 * 128, 64, sx)
            nc.vector.tensor_copy(out=vT2[:, ss], in_=pt[0:64, :])

        def v_transpose(sx):
            # PE transposes (through the proj bank, idle in strip 0) + DVE
            # scatter into the (v | ones) groups
            for tb in range(4 * sx, 4 * sx + 4):
                tbs = slice(tb * 128, (tb + 1) * 128)
                pp = prps.tile([128, IS], F32, name="pp", tag="pp")
                nc.tensor.transpose(pp[:, 0:128], vT01[:, tbs], ident)
                nc.tensor.transpose(pp[:, 128:192], vT2[:, tbs], ident[0:64, 0:64])
                dst01 = v_sb[:, tb, :].rearrange("p (h x) -> p h x", h=3)[:, 0:2, 0:64]
                nc.vector.tensor_copy(
                    out=dst01,
                    in_=pp[:, 0:128].rearrange("p (h x) -> p h x", h=2))
                nc.vector.tensor_copy(out=v_sb[:, tb, 256:320], in_=pp[:, 128:192])

        q_state = {}

        def q_main(qs):
            q_state["pt"] = qkv_chain(0, 128, qs)

        def q_swap(qs):
            pt_s = qkv_chain(128, 128, qs)
            _rope_tile(nc, rope_pool, q01, q_state["pt"], pt_s, cos_sb, sin_sb, qs)

        s0_fillers = [
            lambda: k_main(1), lambda: k_swap(1),
            lambda: qk2_main(1), lambda: qk2_swap(1),
            lambda: k_main(2), lambda: k_swap(2),
            lambda: qk2_main(2), lambda: qk2_swap(2),
            lambda: k_main(3), lambda: k_swap(3),
            lambda: qk2_main(3), lambda: qk2_swap(3),
            lambda: q_main(1), lambda: q_swap(1),
            lambda: v_chain01(0), lambda: v_chain2(0),
            lambda: q_main(2), lambda: q_swap(2),
            lambda: v_chain01(1), lambda: v_chain2(1),
            lambda: q_main(3), lambda: q_swap(3),
            lambda: (v_chain01(2), v_chain2(2)),
            lambda: (v_chain01(3), v_chain2(3)),
        ]
        fi = 0
        for jb in range(NJB):
            score01(0, jb)
            if jb >= 1:
                score_h2(0, jb - 1)
            if jb == NJB - 1:
                score_h2(0, jb)
            # one filler per group while the k chains race the scores;
            # two per group in the back half so nothing spills past the
            # strip into ScalarE-idle serial time
            for _ in range(1 if jb < 8 else 2):
                if fi < len(s0_fillers):
                    s0_fillers[fi]()
                    fi += 1
        while fi < len(s0_fillers):
            s0_fillers[fi]()
            fi += 1
        ph1_stack.close()

        # ---- strips 1..3 + projections + tail ----------------------------
        with (
            tc.tile_pool(name="attnB", bufs=1) as attnB,
            tc.tile_pool(name="nrm", bufs=2) as nrm,
            tc.tile_pool(name="prout", bufs=4) as prout,
        ):
            e01[1] = attnB.tile([128, NJB, 2, ISA], V_DT, name="e01_1")
            e2[1] = attnB.tile([128, NJB, ISA], V_DT, name="e2_1")

            def norm01_copies(s, pv0, pv1):
                """Copy both heads' PV out of PSUM, packed for one recip:
                rows 0:64 = h0, rows 64:128 = h1 (frees both banks fast)."""
                c01n = nrm.tile([128, ISA], F32, name="c01n", tag="c01n")
                c01d = nrm.tile([128, ISA], F32, name="c01d", tag="c01d")
                nc.vector.tensor_copy(out=c01n[0:64, :], in_=pv0[0:64, :])
                nc.vector.tensor_copy(out=c01d[0:64, :], in_=pv0[64:128, :])
                nc.vector.tensor_copy(out=c01n[64:128, :], in_=pv1[0:64, :])
                nc.vector.tensor_copy(out=c01d[64:128, :], in_=pv1[64:128, :])
                return c01n, c01d

            def norm01_div(s, c01n, c01d, use_act=False):
                ss = slice(s * ISA, (s + 1) * ISA)
                r01 = nrm.tile([128, ISA], F32, name="r01", tag="r01")
                if use_act:
                    # tail only: ScalarE is idle after the last exp, and Ln
                    # shares a table set with Exp -> 1/L = exp(-ln(L))
                    tl = nrm.tile([128, ISA], F32, name="tl", tag="tl")
                    nc.scalar.activation(out=tl, in_=c01d,
                                         func=mybir.ActivationFunctionType.Ln)
                    nc.scalar.activation(out=r01, in_=tl, scale=-1.0,
                                         func=mybir.ActivationFunctionType.Exp)
                else:
                    nc.vector.reciprocal(r01, c01d)
                nc.vector.tensor_mul(out=P0[0:64, ss], in0=c01n[0:64, :],
                                     in1=r01[0:64, :])
                nc.vector.tensor_mul(out=P0[64:128, ss], in0=c01n[64:128, :],
                                     in1=r01[64:128, :])

            def norm2(s, pv, use_act=False):
                ss = slice(s * ISA, (s + 1) * ISA)
                c2n = nrm.tile([64, ISA], F32, name="c2n", tag="c2n")
                c2d = nrm.tile([64, ISA], F32, name="c2d", tag="c2d")
                nc.vector.tensor_copy(out=c2n, in_=pv[0:64, :])
                nc.vector.tensor_copy(out=c2d, in_=pv[64:128, :])
                r2 = nrm.tile([64, ISA], F32, name="r2", tag="r2")
                if use_act:
                    t2 = nrm.tile([64, ISA], F32, name="t2", tag="t2")
                    nc.scalar.activation(out=t2, in_=c2d,
                                         func=mybir.ActivationFunctionType.Ln)
                    nc.scalar.activation(out=r2, in_=t2, scale=-1.0,
                                         func=mybir.ActivationFunctionType.Exp)
                else:
                    nc.vector.reciprocal(r2, c2d)
                nc.vector.tensor_mul(out=P1[0:64, ss], in0=c2n, in1=r2)
                nc.vector.tensor_copy(out=P1[64:128, ss], in_=P1[0:64, ss])

            def proj_obs(s, obs):
                ss = slice(s * IS, (s + 1) * IS)
                for ob in obs:
                    obsl = slice(ob * 128, (ob + 1) * 128)
                    pp = prps.tile([128, IS], F32, name="pp", tag="pp")
                    nc.tensor.matmul(pp, wp_sb[:, 0, obsl], P0[:, ss],
                                     start=True, stop=False)
                    nc.tensor.matmul(pp, wp_sb[:, 1, obsl], P1[:, ss],
                                     start=False, stop=True)
                    ot = prout.tile([128, IS], O_DT, name="ot", tag="ot")
                    nc.vector.tensor_copy(out=ot, in_=pp)
                    nc.sync.dma_start(outT[obsl, ss], ot)

            pvst = {}

            def pv_start(ps, h):
                pv = wkps.tile([128, ISA], F32, name="wk", tag="wk")
                pvst[(ps, h)] = pv
                pv_mms(ps, h, pv, range(0, 8))

            def pv_end(ps, h):
                pv_mms(ps, h, pvst[(ps, h)], range(8, NJB))

            # strips 1..3: PV of strip s-1 (heads sequential, 2-bank
            # rotation) + norms + projections of earlier strips as filler
            nstate = {}
            for s in range(1, NSA):
                ps = s - 1

                def n_copies(ps=ps):
                    nstate["c"] = norm01_copies(ps, pvst.pop((ps, 0)),
                                                pvst.pop((ps, 1)))

                def n_div(ps=ps):
                    norm01_div(ps, *nstate.pop("c"))

                fillers = []
                if s == 1:
                    fillers += [lambda sx=sx: v_transpose(sx) for sx in range(4)]
                fillers += [
                    lambda ps=ps: pv_start(ps, 0), lambda ps=ps: pv_end(ps, 0),
                    lambda ps=ps: pv_start(ps, 1), lambda ps=ps: pv_end(ps, 1),
                    n_copies,
                    lambda ps=ps: pv_start(ps, 2), lambda ps=ps: pv_end(ps, 2),
                    n_div,
                    lambda ps=ps: norm2(ps, pvst.pop((ps, 2))),
                ]
                if s >= 2:
                    fillers += [lambda ob=ob, t=s - 2: proj_obs(t, [2 * ob, 2 * ob + 1])
                                for ob in range(3)]
                if s == NSA - 1:
                    fillers += [lambda ob=ob, t=s - 1: proj_obs(t, [2 * ob, 2 * ob + 1])
                                for ob in range(3)]
                    # start the last strip's h0 PV early (its exps are done
                    # through jb14 by the final group; bank freed by the
                    # norm copies above)
                    def pv30():
                        pv = wkps.tile([128, ISA], F32, name="wk", tag="wk")
                        pvst[(s, 0)] = pv
                        pv_mms(s, 0, pv, range(0, 15))
                    fillers.append(pv30)
                fi = 0
                for jb in range(NJB):
                    score01(s, jb)
                    if jb >= 1:
                        score_h2(s, jb - 1)
                    if jb == NJB - 1:
                        score_h2(s, jb)
                    if fi < len(fillers):
                        fillers[fi]()
                        fi += 1
                while fi < len(fillers):
                    fillers[fi]()
                    fi += 1

            # tail: finish the last strip
            s = NSA - 1
            pv_mms(s, 0, pvst[(s, 0)], [15])
            pv_start(s, 1)
            pv_end(s, 1)
            c = norm01_copies(s, pvst.pop((s, 0)), pvst.pop((s, 1)))
            pv_start(s, 2)
            pv_end(s, 2)
            norm01_div(s, *c)
            norm2(s, pvst.pop((s, 2)))
            proj_obs(s, range(0, 6))


# ---------------------------------------------------------------------------
# Host-side sharding / unsharding
# ---------------------------------------------------------------------------

def _rope_tables():
    inv_freq = 1.0 / (ROPE_THETA ** (np.arange(0, D, 2, dtype=np.float64) / D))
    ang = np.arange(N, dtype=np.float64)[None, :] * inv_freq[:, None]  # [32, N]
    cos64 = np.concatenate([np.cos(ang), np.cos(ang)], axis=0)
    sin64 = np.concatenate([-np.sin(ang), np.sin(ang)], axis=0)
    cosT = np.concatenate([cos64, cos64], axis=0).astype(np.float32)
    sinT = np.concatenate([sin64, sin64], axis=0).astype(np.float32)
    return cosT, sinT


def _conv(a, dt):
    """Convert fp32 array for a device tensor of dtype dt."""
    import ml_dtypes

    a = np.ascontiguousarray(a, dtype=np.float32)
    return a.astype(ml_dtypes.bfloat16) if dt == BF16 else a


def make_core_inputs(x, w_qkv, w_proj):
    """Build the 8 per-core input dicts from full inputs."""
    x = np.asarray(x, dtype=np.float32)
    w_qkv = np.asarray(w_qkv, dtype=np.float32)
    w_proj = np.asarray(w_proj, dtype=np.float32)

    cosT, sinT = _rope_tables()
    perm = np.concatenate([np.arange(0, D, 2), np.arange(1, D, 2)])  # de-interleave
    wq, wk, wv = w_qkv[0:C], w_qkv[C : 2 * C], w_qkv[2 * C : 3 * C]
    scale = np.float32(D ** -0.5)
    wpT = np.ascontiguousarray(w_proj.T)  # [in_features, out_channels]

    in_maps = []
    for c in range(NCORES):
        b, g = divmod(c, 4)
        h0, h1, h2 = 3 * g, 3 * g + 1, 3 * g + 2

        def qrow(h):
            return wq[h * D : (h + 1) * D][perm] * scale

        def krow(h):
            return wk[h * D : (h + 1) * D][perm]

        def vrow(h):
            return wv[h * D : (h + 1) * D]

        def swap32(w64):
            # rows permuted by the rotate-half partner p ^ 32
            return np.concatenate([w64[32:64], w64[0:32]], axis=0)

        blocks = [qrow(h0), qrow(h1)]
        blocks += [swap32(qrow(h0)), swap32(qrow(h1))]
        blocks += [krow(h0), krow(h1)]
        blocks += [swap32(krow(h0)), swap32(krow(h1))]
        blocks += [qrow(h2), krow(h2)]
        blocks += [swap32(qrow(h2)), swap32(krow(h2))]
        blocks += [vrow(h0), vrow(h1), vrow(h2)]
        w_feat = np.concatenate(blocks, axis=0).T  # [C, 15*D]
        wp_rows = np.concatenate(
            [wpT[h0 * D : (h0 + 1) * D], wpT[h1 * D : (h1 + 1) * D],
             0.5 * wpT[h2 * D : (h2 + 1) * D], 0.5 * wpT[h2 * D : (h2 + 1) * D]],
            axis=0,
        )  # [256, C]
        in_maps.append(
            {
                "xT": np.ascontiguousarray(x[b].T),
                "w_feat": np.ascontiguousarray(w_feat, dtype=np.float32),
                "wp": _conv(wp_rows, P_DT),
                "cosT": cosT,
                "sinT": sinT,
                "ones": _conv(np.ones((128, D), dtype=np.float32), V_DT),
            }
        )
    return in_maps


def unshard(core_outs, b_proj):
    """Sum the 4 partial projections per batch, transpose, add bias."""
    b_proj = np.asarray(b_proj, dtype=np.float32)
    out = np.empty((B, N, C), dtype=np.float32)
    for b in range(B):
        acc = np.asarray(core_outs[4 * b], dtype=np.float32).copy()
        for g in range(1, 4):
            acc += np.asarray(core_outs[4 * b + g], dtype=np.float32)
        out[b] = acc.T + b_proj
    return out


_NC_CACHE = {}


def get_nc():
    key = (MM_DT, QK_DT, V_DT, P_DT, O_DT, DEBUG_DUMP)
    if key not in _NC_CACHE:
        _NC_CACHE[key] = build_nc()
    return _NC_CACHE[key]


def run(inputs, trace=False, **spmd_kwargs):
    """Run on hardware; returns (full_output, BassKernelResults)."""
    nc = get_nc()
    in_maps = make_core_inputs(inputs["x"], inputs["w_qkv"], inputs["w_proj"])
    res = bass_utils.run_bass_kernel_spmd(
        nc, in_maps, core_ids=list(range(NCORES)), trace=trace, **spmd_kwargs
    )
    core_outs = [r["outT"] for r in res.results]
    return unshard(core_outs, inputs["b_proj"]), res


def kernel(x, w_qkv, w_proj, b_proj):
    out, _ = run({"x": x, "w_qkv": w_qkv, "w_proj": w_proj, "b_proj": b_proj})
    return out



# revision 5
# speedup vs baseline: 1.0391x; 1.0391x over previous
"""Trainium2 Bass kernel for nn_Attention_7602092114471.

Full multi-head attention block:
  qkv = x @ w_qkv.T ; split q,k,v into 12 heads of d=64
  q = rope(q * d**-0.5) ; k = rope(k)   (lucidrains interleaved RoPE)
  attn = softmax(q @ k.T) ; out = (attn @ v) reassembled, @ w_proj.T + b_proj

Shapes: x [2, 2048, 768], w_qkv [2304, 768], w_proj [768, 768], b_proj [768].

Sharding: 24 (batch, head) pairs -> 8 cores x 3 heads. Core c handles batch
c//4, heads {3g, 3g+1, 3g+2} with g = c%4. Each core computes its heads'
q/k/v projections, attention, and a partial output projection over its
3 heads' feature columns. The host sums the 4 partial projections per batch
(the tensor-parallel all-reduce, done on host during unshard) and adds bias.

v2 design (from the 218us v1 trace: PE busy 167us of which 82us was fp32r
projections, ScalarE 115us = 128 exp ACTs, DVE 115us incl 27us reciprocals):
  * All matmuls bf16 (x, w cast on host). fp32r ran at 2 cycles/col.
  * No swap-projection chains: rope's rotate-half-by-32 partner comes from
    4 cross-partition [32,512] bf16 SBUF copies (DVE 4x rate) off a bf16
    copy of the projection PSUM; sin sign pattern baked in the host table.
  * v produced token-major directly: per 128-token block, 6 accumulating
    MMs with x-block stationary and the v weight columns moving ->
    [tok, 3*64] PSUM, one DVE scatter into the (v | ones) groups. No PE
    transposes, ones prefilled by one gpsimd memset.
  * Scores: one [128, 3, 512] PSUM group (3 banks, double-buffered) per
    key block = 3 MMs + ONE 1536-col exp ACT for all 3 heads (v1 paid 2
    ACTs' fixed overhead per block). PSUM: 6 + 2 (chains/PV/proj) = 8.
  * Softmax 1/L: h0/h1 via ScalarE Ln(PSUM)->Exp(-x) (same table set as
    Exp, so no table switch); h2 via DVE reciprocal; normalization muls
    read PV numerators straight from PSUM (no copies).
  * PE HAM warmup (dummy MMs) + early exp-table load during input DMA;
    prologue-ordered DMA (k/qk2/q01 weight blocks + x strip 0 first).
  * Tail: strip 3's h0 PV steals the retiring score-group PSUM buffer and
    runs during the last score groups; norm2 via ScalarE in the tail.
"""

import os

import numpy as np

import concourse.bass as bass
import concourse.mybir as mybir
import concourse.tile as tile
from concourse import bacc, bass_utils

# Problem constants (hardcoded per contract; kernel.py must be self-contained).
B = 2
N = 2048
C = 768
H = 12
D = 64
ROPE_THETA = 10000.0
NCORES = 8

F32 = mybir.dt.float32
BF16 = mybir.dt.bfloat16

IS = 512                  # strip width (projections and attention i-strips)
NSTRIP = N // IS          # 4
NJB = N // 128            # 16 key blocks
KT = C // 128             # 6 contraction tiles for the projections
EXP_BIAS = -8.0           # constant shift inside exp; cancels in normalization

# w_feat column blocks
Q01 = 0                   # q0|q1 (scaled, de-interleaved rows)
K01 = 128                 # k0|k1
QK2 = 256                 # q2|k2
V012 = 384                # v0|v1|v2 (192 cols)
WF = 576

DEBUG_DUMP = os.environ.get("K_DEBUG_DUMP", "0") == "1"
WARM_MMS = int(os.environ.get("K_WARM_MMS", "40"))


def build_nc():
    """Build the per-core Bass module (same NEFF runs SPMD on all 8 cores)."""
    nc = bacc.Bacc(
        "TRN2",
        target_bir_lowering=False,
        debug=False,
        enable_asserts=False,
    )

    xT = nc.dram_tensor("xT", [C, N], BF16, kind="ExternalInput").ap()
    w_feat = nc.dram_tensor("w_feat", [C, WF], BF16, kind="ExternalInput").ap()
    wp = nc.dram_tensor("wp", [256, C], BF16, kind="ExternalInput").ap()
    cosT = nc.dram_tensor("cosT", [128, N], BF16, kind="ExternalInput").ap()
    sinT = nc.dram_tensor("sinT", [128, N], BF16, kind="ExternalInput").ap()
    outT = nc.dram_tensor("outT", [C, N], BF16, kind="ExternalOutput").ap()
    dbg = None
    if DEBUG_DUMP:
        dbg = {
            nm: nc.dram_tensor(f"dbg_{nm}", shp, dt, kind="ExternalOutput").ap()
            for nm, shp, dt in [
                ("q01", [128, N], BF16), ("k01", [128, N], BF16),
                ("qk2d", [128, N], BF16),
                ("v_sb", [128, NJB * 384], BF16),
                ("e0", [128, NJB * 3 * IS], BF16),
                ("e1", [128, NJB * 3 * IS], BF16),
                ("P0", [128, N], BF16), ("P1", [128, N], BF16),
            ]
        }

    with tile.TileContext(nc) as tc:
        _kernel_body(tc, nc, xT, w_feat, wp, cosT, sinT, outT, dbg)
    nc.compile()
    return nc


def _kernel_body(tc, nc, xT, w_feat, wp, cosT, sinT, outT, dbg=None):
    import contextlib

    Exp = mybir.ActivationFunctionType.Exp
    Ln = mybir.ActivationFunctionType.Ln

    ctx = contextlib.ExitStack()
    with ctx:
        persist = ctx.enter_context(tc.tile_pool(name="persist", bufs=1))
        rope_pool = ctx.enter_context(tc.tile_pool(name="rope", bufs=2))
        nrm = ctx.enter_context(tc.tile_pool(name="nrm", bufs=2))
        prout = ctx.enter_context(tc.tile_pool(name="prout", bufs=4))
        attnA = ctx.enter_context(tc.tile_pool(name="attnA", bufs=1))
        # PSUM: 3-bank score groups double-buffered (6) + 2 work banks
        stsp = ctx.enter_context(tc.tile_pool(name="sts", bufs=2, space="PSUM"))
        wkps = ctx.enter_context(tc.tile_pool(name="wkps", bufs=2, space="PSUM"))

        # ---- persistent SBUF tensors -------------------------------------
        q01 = persist.tile([128, N], BF16, name="q01")
        k01 = persist.tile([128, N], BF16, name="k01")
        qk2d = persist.tile([128, N], BF16, name="qk2d")  # q2 rows 0:64 | k2 64:128
        k2lo = persist.tile([64, N], BF16, name="k2lo")   # k2 at base partition 0
        v_sb = persist.tile([128, NJB, 3, 128], BF16, name="v_sb")
        P0 = persist.tile([128, N], BF16, name="P0")  # heads h0 | h1
        P1 = persist.tile([128, N], BF16, name="P1")  # h2 duplicated
        wp_sb = persist.tile([128, 2, C], BF16, name="wp_sb")
        bias_sb = persist.tile([128, 1], F32, name="bias_sb")
        warm = persist.tile([128, 64], F32, name="warm")
        warm_o = persist.tile([128, 64], F32, name="warm_o")

        nc.vector.memset(bias_sb, EXP_BIAS)
        nc.vector.memset(warm, 0.0)
        # early ACT table load (Exp+Ln share natural_log_exp_and_others)
        nc.scalar.activation(out=warm_o, in_=warm, func=Exp)
        # ones columns of the (v | ones) PV groups
        nc.gpsimd.memset(v_sb[:, :, :, 64:128], 1.0)

        e_all = [attnA.tile([128, NJB, 3, IS], BF16, name="e0"), None]

        ph1_stack = contextlib.ExitStack()
        ph1 = ph1_stack.enter_context(tc.tile_pool(name="ph1", bufs=1))
        w_sb = ph1.tile([128, KT, WF], BF16, name="w_sb")
        cos_sb = ph1.tile([128, N], BF16, name="cos_sb")
        sin_sb = ph1.tile([128, N], BF16, name="sin_sb")
        x_sb = [
            ph1.tile([128, N], BF16, name=f"x_sb{kt}", tag=f"x_sb{kt}")
            for kt in range(KT)
        ]

        wr = w_feat.rearrange("(o p) f -> p o f", p=128)

        # prologue-critical DMAs first: k01/qk2 weight blocks + x strip 0
        for kt in range(KT):
            nc.sync.dma_start(w_sb[:, kt, K01:K01 + 128], wr[:, kt, K01:K01 + 128])
            nc.sync.dma_start(x_sb[kt][:, 0:IS], xT[kt * 128:(kt + 1) * 128, 0:IS])
        nc.sync.dma_start(cos_sb[:, 0:IS], cosT[:, 0:IS])
        nc.sync.dma_start(sin_sb[:, 0:IS], sinT[:, 0:IS])
        for kt in range(KT):
            nc.sync.dma_start(w_sb[:, kt, QK2:QK2 + 128], wr[:, kt, QK2:QK2 + 128])
        for kt in range(KT):
            nc.sync.dma_start(w_sb[:, kt, Q01:Q01 + 128], wr[:, kt, Q01:Q01 + 128])
        nc.sync.dma_start(cos_sb[:, IS:N], cosT[:, IS:N])
        nc.sync.dma_start(sin_sb[:, IS:N], sinT[:, IS:N])
        for s in range(1, NSTRIP):
            ss = slice(s * IS, (s + 1) * IS)
            for kt in range(KT):
                nc.sync.dma_start(x_sb[kt][:, ss], xT[kt * 128:(kt + 1) * 128, ss])
        for kt in range(KT):
            nc.sync.dma_start(w_sb[:, kt, V012:V012 + 192], wr[:, kt, V012:V012 + 192])
        nc.sync.dma_start(wp_sb, wp.rearrange("(o p) f -> p o f", p=128))

        # PE HAM warmup: dummy zero-MMs keep the PE busy through the DMA
        # window so the clock gate is released before the first real chain
        if WARM_MMS:
            wt = wkps.tile([128, IS], F32, name="wk", tag="wk")
            for _ in range(WARM_MMS):
                nc.tensor.matmul(wt[:64, 0:64], warm[:, 0:64], warm[:, 0:64],
                                 start=True, stop=True)

        # ---- projection chain + rope ------------------------------------
        def rope_group(dst, colblk, s, pre_scalar=False):
            """One 128-feature projection chain + rope into dst[:, strip s].

            rotate-half partner via 4 cross-partition bf16 copies; sin sign
            pattern ([-sin;+sin] per 32-row half) baked into sinT."""
            ss = slice(s * IS, (s + 1) * IS)
            pt = wkps.tile([128, IS], F32, name="wk", tag="wk")
            for kt in range(KT):
                nc.tensor.matmul(
                    pt, w_sb[:, kt, colblk:colblk + 128], x_sb[kt][:, ss],
                    start=(kt == 0), stop=(kt == KT - 1),
                )
            qpre = rope_pool.tile([128, IS], BF16, name="qpre", tag="qpre")
            if pre_scalar:
                nc.scalar.copy(out=qpre, in_=pt)
            else:
                nc.vector.tensor_copy(out=qpre, in_=pt)
            qps = rope_pool.tile([128, IS], BF16, name="qps", tag="qps")
            for (a, b) in ((0, 32), (32, 0), (64, 96), (96, 64)):
                nc.vector.tensor_copy(out=qps[a:a + 32, :], in_=qpre[b:b + 32, :])
            tmp1 = rope_pool.tile([128, IS], BF16, name="tmp1", tag="tmp1")
            tmp2 = rope_pool.tile([128, IS], BF16, name="tmp2", tag="tmp2")
            nc.vector.tensor_mul(out=tmp1, in0=qpre, in1=cos_sb[:, ss])
            nc.vector.tensor_mul(out=tmp2, in0=qps, in1=sin_sb[:, ss])
            nc.vector.tensor_add(out=dst[:, ss], in0=tmp1, in1=tmp2)
            if dst is qk2d:
                # matmul needs lhsT/rhs on the same base partition: keep a
                # base-0 copy of k2 for the h2 score matmuls
                nc.vector.tensor_copy(out=k2lo[:, ss], in_=qk2d[64:128, ss])

        def v_block(tb):
            """v for one 128-token block, token-major: x-block stationary,
            v weight columns moving -> [tok, 3*64] -> scatter into v_sb."""
            tbs = slice(tb * 128, (tb + 1) * 128)
            kt0 = 0
            pt = wkps.tile([128, IS], F32, name="wk", tag="wk")
            for kt in range(KT):
                nc.tensor.matmul(
                    pt[:, 0:192], x_sb[kt][:, tbs], w_sb[:, kt, V012:V012 + 192],
                    start=(kt == 0), stop=(kt == KT - 1),
                )
            nc.vector.tensor_copy(
                out=v_sb[:, tb, :, 0:64],
                in_=pt[:, 0:192].rearrange("p (h x) -> p h x", h=3),
            )

        # ---- scores + exp -----------------------------------------------
        def score_group(s, jb):
            ss = slice(s * IS, (s + 1) * IS)
            jbs = slice(jb * 128, (jb + 1) * 128)
            st = stsp.tile([128, 3, IS], F32, name="st", tag="st")
            nc.tensor.matmul(st[:, 0, :], k01[0:64, jbs], q01[0:64, ss],
                             start=True, stop=True)
            nc.tensor.matmul(st[:, 1, :], k01[64:128, jbs], q01[64:128, ss],
                             start=True, stop=True)
            nc.tensor.matmul(st[:, 2, :], k2lo[:, jbs], qk2d[0:64, ss],
                             start=True, stop=True)
            nc.scalar.activation(
                out=e_all[s % 2][:, jb], in_=st,
                func=Exp, bias=bias_sb[:, :],
            )

        # ---- PV + normalization -----------------------------------------
        pvst = {}

        def pv_mms(ps, h, pv, jbs):
            et = e_all[ps % 2]
            for jb in jbs:
                nc.tensor.matmul(
                    pv, v_sb[:, jb, h, :], et[:, jb, h, :],
                    start=(jb == 0), stop=(jb == NJB - 1),
                )

        def pv_start(ps, h):
            pv = wkps.tile([128, IS], F32, name="wk", tag="wk")
            pvst[(ps, h)] = pv
            pv_mms(ps, h, pv, range(0, 8))

        def pv_end(ps, h):
            pv_mms(ps, h, pvst[(ps, h)], range(8, NJB))

        def norm01(ps):
            """h0/h1: 1/L on ScalarE (Ln from PSUM, then Exp(-x); same table
            set as the score exps); numerator muls read PV PSUM directly."""
            ss = slice(ps * IS, (ps + 1) * IS)
            pv0 = pvst.pop((ps, 0))
            pv1 = pvst.pop((ps, 1))
            lnt = nrm.tile([128, IS], F32, name="lnt", tag="lnt")
            nc.scalar.activation(out=lnt[0:64, :], in_=pv0[64:128, :], func=Ln)
            nc.scalar.activation(out=lnt[64:128, :], in_=pv1[64:128, :], func=Ln)
            r01 = nrm.tile([128, IS], F32, name="r01", tag="r01")
            nc.scalar.activation(out=r01, in_=lnt, func=Exp, scale=-1.0)
            nc.vector.tensor_mul(out=P0[0:64, ss], in0=pv0[0:64, :],
                                 in1=r01[0:64, :])
            nc.vector.tensor_mul(out=P0[64:128, ss], in0=pv1[0:64, :],
                                 in1=r01[64:128, :])

        def norm2(ps, use_act=False):
            ss = slice(ps * IS, (ps + 1) * IS)
            pv2 = pvst.pop((ps, 2))
            r2 = nrm.tile([64, IS], F32, name="r2", tag="r2")
            if use_act:
                # tail only: ScalarE is idle after the last exp
                t2 = nrm.tile([64, IS], F32, name="t2", tag="t2")
                nc.scalar.activation(out=t2, in_=pv2[64:128, :], func=Ln)
                nc.scalar.activation(out=r2, in_=t2, scale=-1.0, func=Exp)
            else:
                nc.vector.reciprocal(r2, pv2[64:128, :])
            nc.vector.tensor_mul(out=P1[0:64, ss], in0=pv2[0:64, :], in1=r2)
            nc.vector.tensor_copy(out=P1[64:128, ss], in_=P1[0:64, ss])

        def proj_obs(t, obs):
            ss = slice(t * IS, (t + 1) * IS)
            for ob in obs:
                obsl = slice(ob * 128, (ob + 1) * 128)
                pp = wkps.tile([128, IS], F32, name="wk", tag="wk")
                nc.tensor.matmul(pp, wp_sb[:, 0, obsl], P0[:, ss],
                                 start=True, stop=False)
                nc.tensor.matmul(pp, wp_sb[:, 1, obsl], P1[:, ss],
                                 start=False, stop=True)
                ot = prout.tile([128, IS], BF16, name="ot", tag="ot")
                nc.vector.tensor_copy(out=ot, in_=pp)
                nc.sync.dma_start(outT[obsl, ss], ot)

        # ---- prologue: strip 0's own projections ------------------------
        rope_group(k01, K01, 0, pre_scalar=True)
        rope_group(qk2d, QK2, 0, pre_scalar=True)
        rope_group(q01, Q01, 0, pre_scalar=True)

        # ---- strip 0: scores/exp with the rest of phase 1 as filler ------
        # k01/qk2 of strip t must land before score group jb=4t.
        s0_fillers = [
            [lambda: rope_group(k01, K01, 1)],
            [lambda: rope_group(qk2d, QK2, 1)],
            [lambda: rope_group(q01, Q01, 1)],
            [lambda: v_block(0)],
            [lambda: rope_group(k01, K01, 2)],
            [lambda: rope_group(qk2d, QK2, 2)],
            [lambda: rope_group(q01, Q01, 2)],
            [lambda: v_block(1)],
            [lambda: rope_group(k01, K01, 3)],
            [lambda: rope_group(qk2d, QK2, 3)],
            [lambda: rope_group(q01, Q01, 3), lambda: v_block(2)],
            [lambda: v_block(3), lambda: v_block(4)],
            [lambda: v_block(5), lambda: v_block(6)],
            [lambda: v_block(7), lambda: v_block(8), lambda: v_block(9)],
            [lambda: v_block(10), lambda: v_block(11), lambda: v_block(12)],
            [lambda: v_block(13), lambda: v_block(14), lambda: v_block(15)],
        ]
        for jb in range(NJB):
            score_group(0, jb)
            for f in s0_fillers[jb]:
                f()
        ph1_stack.close()

        # ---- strips 1..3 + PV/norm/proj fillers + tail -------------------
        with tc.tile_pool(name="attnB", bufs=1) as attnB:
            e_all[1] = attnB.tile([128, NJB, 3, IS], BF16, name="e1")

            strip_fillers = {
                1: [
                    lambda: pv_start(0, 0), lambda: pv_end(0, 0),
                    lambda: pv_start(0, 1), lambda: pv_end(0, 1),
                    lambda: norm01(0),
                    lambda: pv_start(0, 2), lambda: pv_end(0, 2),
                    lambda: norm2(0),
                ],
                2: [
                    lambda: pv_start(1, 0), lambda: pv_end(1, 0),
                    lambda: pv_start(1, 1), lambda: pv_end(1, 1),
                    lambda: norm01(1),
                    lambda: pv_start(1, 2), lambda: pv_end(1, 2),
                    lambda: norm2(1),
                    lambda: proj_obs(0, [0, 1]),
                    lambda: proj_obs(0, [2, 3]),
                    lambda: proj_obs(0, [4, 5]),
                ],
                3: [
                    lambda: pv_start(2, 0), lambda: pv_end(2, 0),
                    lambda: pv_start(2, 1), lambda: pv_end(2, 1),
                    lambda: norm01(2),
                    lambda: pv_start(2, 2), lambda: pv_end(2, 2),
                    lambda: norm2(2),
                    lambda: proj_obs(1, [0, 1]),
                    lambda: proj_obs(1, [2, 3]),
                    lambda: proj_obs(1, [4, 5]),
                    lambda: proj_obs(2, [0, 1]),
                    lambda: proj_obs(2, [2, 3]),
                    lambda: proj_obs(2, [4, 5]),
                ],
            }
            for s in range(1, NSTRIP):
                fillers = strip_fillers[s]
                fi = 0
                for jb in range(NJB):
                    score_group(s, jb)
                    if fi < len(fillers):
                        fillers[fi]()
                        fi += 1
                while fi < len(fillers):
                    fillers[fi]()
                    fi += 1

            # tail: strip 3's PV. h0 steals the score-group PSUM buffer that
            # retires after ACT(14), so it overlaps the last score groups.
            st3 = stsp.tile([128, 3, IS], F32, name="st", tag="st")
            pv30 = st3[:, 0, :]
            pvst[(3, 0)] = pv30
            pv_mms(3, 0, pv30, range(NJB))
            pv_start(3, 1)
            pv_end(3, 1)
            norm01(3)
            pv_start(3, 2)
            pv_end(3, 2)
            norm2(3, use_act=True)
            proj_obs(3, [0, 1])
            proj_obs(3, [2, 3])
            proj_obs(3, [4, 5])

        if dbg is not None:
            nc.sync.dma_start(dbg["q01"], q01)
            nc.sync.dma_start(dbg["k01"], k01)
            nc.sync.dma_start(dbg["qk2d"], qk2d)
            nc.sync.dma_start(dbg["v_sb"], v_sb.rearrange("p a b c -> p (a b c)"))
            nc.sync.dma_start(dbg["e0"], e_all[0].rearrange("p a b c -> p (a b c)"))
            nc.sync.dma_start(dbg["e1"], e_all[1].rearrange("p a b c -> p (a b c)"))
            nc.sync.dma_start(dbg["P0"], P0)
            nc.sync.dma_start(dbg["P1"], P1)


# ---------------------------------------------------------------------------
# Host-side sharding / unsharding
# ---------------------------------------------------------------------------

def _rope_tables():
    inv_freq = 1.0 / (ROPE_THETA ** (np.arange(0, D, 2, dtype=np.float64) / D))
    ang = np.arange(N, dtype=np.float64)[None, :] * inv_freq[:, None]  # [32, N]
    cos64 = np.concatenate([np.cos(ang), np.cos(ang)], axis=0)
    sin64 = np.concatenate([-np.sin(ang), np.sin(ang)], axis=0)
    cosT = np.concatenate([cos64, cos64], axis=0)
    sinT = np.concatenate([sin64, sin64], axis=0)
    return cosT, sinT  # [128, N] float64


def _bf(a):
    import ml_dtypes

    return np.ascontiguousarray(a).astype(ml_dtypes.bfloat16)


def make_core_inputs(x, w_qkv, w_proj):
    """Build the 8 per-core input dicts from full inputs."""
    x = np.asarray(x, dtype=np.float32)
    w_qkv = np.asarray(w_qkv, dtype=np.float32)
    w_proj = np.asarray(w_proj, dtype=np.float32)

    cosT, sinT = _rope_tables()
    cosT, sinT = _bf(cosT), _bf(sinT)
    perm = np.concatenate([np.arange(0, D, 2), np.arange(1, D, 2)])  # de-interleave
    wq, wk, wv = w_qkv[0:C], w_qkv[C: 2 * C], w_qkv[2 * C: 3 * C]
    scale = np.float32(D ** -0.5)
    wpT = np.ascontiguousarray(w_proj.T)  # [in_features, out_channels]

    in_maps = []
    for c in range(NCORES):
        b, g = divmod(c, 4)
        h0, h1, h2 = 3 * g, 3 * g + 1, 3 * g + 2

        def qrow(h):
            return wq[h * D: (h + 1) * D][perm] * scale

        def krow(h):
            return wk[h * D: (h + 1) * D][perm]

        def vrow(h):
            return wv[h * D: (h + 1) * D]

        blocks = [qrow(h0), qrow(h1)]          # Q01
        blocks += [krow(h0), krow(h1)]         # K01
        blocks += [qrow(h2), krow(h2)]         # QK2
        blocks += [vrow(h0), vrow(h1), vrow(h2)]  # V012
        w_feat = np.concatenate(blocks, axis=0).T  # [C, 576]
        wp_rows = np.concatenate(
            [wpT[h0 * D: (h0 + 1) * D], wpT[h1 * D: (h1 + 1) * D],
             0.5 * wpT[h2 * D: (h2 + 1) * D], 0.5 * wpT[h2 * D: (h2 + 1) * D]],
            axis=0,
        )  # [256, C]
        in_maps.append(
            {
                "xT": _bf(x[b].T),
                "w_feat": _bf(w_feat),
                "wp": _bf(wp_rows),
                "cosT": cosT,
                "sinT": sinT,
            }
        )
    return in_maps


def unshard(core_outs, b_proj):
    """Sum the 4 partial projections per batch, transpose, add bias."""
    b_proj = np.asarray(b_proj, dtype=np.float32)
    out = np.empty((B, N, C), dtype=np.float32)
    for b in range(B):
        acc = np.asarray(core_outs[4 * b], dtype=np.float32).copy()
        for g in range(1, 4):
            acc += np.asarray(core_outs[4 * b + g], dtype=np.float32)
        out[b] = acc.T + b_proj
    return out


_NC_CACHE = {}


def get_nc():
    key = (DEBUG_DUMP, WARM_MMS)
    if key not in _NC_CACHE:
        _NC_CACHE[key] = build_nc()
    return _NC_CACHE[key]


def run(inputs, trace=False, **spmd_kwargs):
    """Run on hardware; returns (full_output, BassKernelResults)."""
    nc = get_nc()
    in_maps = make_core_inputs(inputs["x"], inputs["w_qkv"], inputs["w_proj"])
    res = bass_utils.run_bass_kernel_spmd(
        nc, in_maps, core_ids=list(range(NCORES)), trace=trace, **spmd_kwargs
    )
    core_outs = [r["outT"] for r in res.results]
    return unshard(core_outs, inputs["b_proj"]), res


def kernel(x, w_qkv, w_proj, b_proj):
    out, _ = run({"x": x, "w_qkv": w_qkv, "w_proj": w_proj, "b_proj": b_proj})
    return out


# revision 12
# speedup vs baseline: 1.0779x; 1.0373x over previous
"""Trainium2 Bass kernel for nn_Attention_7602092114471.

Full multi-head attention block:
  qkv = x @ w_qkv.T ; split q,k,v into 12 heads of d=64
  q = rope(q * d**-0.5) ; k = rope(k)   (lucidrains interleaved RoPE)
  attn = softmax(q @ k.T) ; out = (attn @ v) reassembled, @ w_proj.T + b_proj

Shapes: x [2, 2048, 768], w_qkv [2304, 768], w_proj [768, 768], b_proj [768].

Sharding: 24 (batch, head) pairs -> 8 cores x 3 heads. Core c handles batch
c//4, heads {3g, 3g+1, 3g+2} with g = c%4. Each core computes its heads'
q/k/v projections, attention, and a partial output projection over its
3 heads' feature columns. The host sums the 4 partial projections per batch
(the tensor-parallel all-reduce, done on host during unshard) and adds bias.

v3 design (v1 218us -> v2 209us -> this):
  * All projection/score matmuls bf16 (fp32r ran 2 cycles/col).
  * PV in fp8e4 DoubleRow: exp ACTs write e as fp8 pairs ([128, 8, 2, 3,
    512] layout), v cast to fp8 in a paired layout, so each PV matmul
    contracts TWO 128-token key blocks in one 512-cycle pass. e and v are
    consistent in the softmax numerator/denominator (ones-columns trick),
    so the fp8 error largely cancels; exp bias is -4 to keep e well above
    the fp8 denormal cutoff.
  * No swap-projection chains: rope's rotate-half partner via 4 cross-
    partition [32,512] bf16 SBUF copies (DVE 4x rate) off a bf16 copy of
    the projection PSUM; sin sign pattern baked into the host table.
  * v produced token-major directly (x-block stationary, v weight columns
    moving) -> [tok, 3*64] PSUM -> one fp8 scatter per block. No PE
    transposes; ones columns prefilled by one gpsimd memset.
  * Scores: one [128, 3, 512] PSUM group (3 banks, double-buffered) per
    key block = 3 MMs + ONE 1536-col exp ACT for all 3 heads.
  * Softmax 1/L: DVE stock reciprocal in-loop (Ln/Exp thrashed the ACT
    table sets -13 reloads in v2); ScalarE Reciprocal ACTs in the tail
    where the single table switch overlaps PE work.
  * DMA ordered for the prologue (k01/qk2 weight blocks, x strip 0 split
    in halves, then the rest); dummy exp ACT at t0 preloads the table.
"""

import os

import numpy as np

import concourse.bass as bass
import concourse.mybir as mybir
import concourse.tile as tile
from concourse import bacc, bass_utils

# Problem constants (hardcoded per contract; kernel.py must be self-contained).
B = 2
N = 2048
C = 768
H = 12
D = 64
ROPE_THETA = 10000.0
NCORES = 8

F32 = mybir.dt.float32
BF16 = mybir.dt.bfloat16
FP8 = mybir.dt.float8e4

IS = 512                  # strip width (projections and attention i-strips)
NSTRIP = N // IS          # 4
NJB = N // 128            # 16 key blocks
NJP = NJB // 2            # 8 key-block pairs (fp8 DoubleRow PV)
KT = C // 128             # 6 contraction tiles for the projections
EXP_BIAS = -2.0           # constant shift inside exp; cancels in normalization.
                          # Chosen for fp8 e: keeps the softmax mass in the
                          # e4m3 normal range (floor 2^-6 = e^-4.2 -> only
                          # scores ~6+ below the max flush) while max
                          # e = e^(5.6-2) ~ 37 stays far from the 240 limit.

# w_feat column blocks
Q01 = 0                   # q0|q1 (scaled, de-interleaved rows)
K01 = 128                 # k0|k1
QK2 = 256                 # q2|k2
V012 = 384                # v0|v1|v2 (192 cols)
WF = 576

DEBUG_DUMP = os.environ.get("K_DEBUG_DUMP", "0") == "1"
PV_FP8 = os.environ.get("K_PV_FP8", "0") == "1"  # ~2.3%% rel err: off
EV_DT = FP8 if PV_FP8 else BF16


def build_nc():
    """Build the per-core Bass module (same NEFF runs SPMD on all 8 cores)."""
    nc = bacc.Bacc(
        "TRN2",
        target_bir_lowering=False,
        debug=False,
        enable_asserts=False,
    )

    xT = nc.dram_tensor("xT", [C, N], BF16, kind="ExternalInput").ap()
    w_feat = nc.dram_tensor("w_feat", [C, WF], BF16, kind="ExternalInput").ap()
    wp = nc.dram_tensor("wp", [256, C], BF16, kind="ExternalInput").ap()
    cosT = nc.dram_tensor("cosT", [128, N], BF16, kind="ExternalInput").ap()
    sinT = nc.dram_tensor("sinT", [128, N], BF16, kind="ExternalInput").ap()
    outT = nc.dram_tensor("outT", [C, N], BF16, kind="ExternalOutput").ap()
    dbg = None
    if DEBUG_DUMP:
        dbg = {
            nm: nc.dram_tensor(f"dbg_{nm}", shp, dt, kind="ExternalOutput").ap()
            for nm, shp, dt in [
                ("q01", [128, N], BF16), ("k01", [128, N], BF16),
                ("qk2d", [128, N], BF16),
                ("v_sb", [128, NJP * 2 * 384], EV_DT),
                ("e0", [128, NJP * 2 * 3 * IS], EV_DT),
                ("e1", [128, NJP * 2 * 3 * IS], EV_DT),
                ("P0", [128, N], BF16), ("P1", [128, N], BF16),
            ]
        }

    with tile.TileContext(nc) as tc:
        _kernel_body(tc, nc, xT, w_feat, wp, cosT, sinT, outT, dbg)
    nc.compile()
    return nc


def _kernel_body(tc, nc, xT, w_feat, wp, cosT, sinT, outT, dbg=None):
    import contextlib

    Exp = mybir.ActivationFunctionType.Exp
    Recip = mybir.ActivationFunctionType.Reciprocal
    DR = mybir.MatmulPerfMode.DoubleRow

    ctx = contextlib.ExitStack()
    with ctx:
        persist = ctx.enter_context(tc.tile_pool(name="persist", bufs=1))
        rope_pool = ctx.enter_context(tc.tile_pool(name="rope", bufs=2))
        nrm = ctx.enter_context(tc.tile_pool(name="nrm", bufs=2))
        prout = ctx.enter_context(tc.tile_pool(name="prout", bufs=4))
        attnA = ctx.enter_context(tc.tile_pool(name="attnA", bufs=1))
        # PSUM: 3-bank score groups double-buffered (6) + 2 work banks
        stsp = ctx.enter_context(tc.tile_pool(name="sts", bufs=2, space="PSUM"))
        wkps = ctx.enter_context(tc.tile_pool(name="wkps", bufs=2, space="PSUM"))

        # ---- persistent SBUF tensors -------------------------------------
        q01 = persist.tile([128, N], BF16, name="q01")
        k01 = persist.tile([128, N], BF16, name="k01")
        qk2d = persist.tile([128, N], BF16, name="qk2d")  # q2 rows 0:64 | k2 64:128
        k2lo = persist.tile([64, N], BF16, name="k2lo")   # k2 at base partition 0
        # (v | ones) stationary groups, fp8, key-block-PAIRED for DoubleRow
        v_sb = persist.tile([128, NJP, 2, 3, 128], EV_DT, name="v_sb")
        P0 = persist.tile([128, N], BF16, name="P0")  # heads h0 | h1
        P1 = persist.tile([128, N], BF16, name="P1")  # h2 duplicated
        wp_sb = persist.tile([128, 2, C], BF16, name="wp_sb")
        bias_sb = persist.tile([128, 1], F32, name="bias_sb")
        warm = persist.tile([128, 64], F32, name="warm")
        warm_o = persist.tile([128, 64], F32, name="warm_o")

        e_all = [
            attnA.tile([128, NJP, 2, 3, IS], EV_DT, name="e0"),
            attnA.tile([128, NJP, 2, 3, IS], EV_DT, name="e1"),
        ]

        ph1_stack = contextlib.ExitStack()
        ph1 = ph1_stack.enter_context(tc.tile_pool(name="ph1", bufs=1))
        w_sb = ph1.tile([128, KT, WF], BF16, name="w_sb")
        cos_sb = ph1.tile([128, N], BF16, name="cos_sb")
        sin_sb = ph1.tile([128, N], BF16, name="sin_sb")
        x_sb = [
            ph1.tile([128, N], BF16, name=f"x_sb{kt}", tag=f"x_sb{kt}")
            for kt in range(KT)
        ]

        wr = w_feat.rearrange("(o p) f -> p o f", p=128)

        # prologue-critical DMAs first: k01/qk2 weight blocks, then x strip 0
        # split in halves so no ring carries more than ~64KB before the
        # prologue chains can start.
        for kt in range(KT):
            nc.sync.dma_start(w_sb[:, kt, K01:K01 + 128], wr[:, kt, K01:K01 + 128])
        for kt in range(KT):
            nc.sync.dma_start(w_sb[:, kt, QK2:QK2 + 128], wr[:, kt, QK2:QK2 + 128])
        for kt in range(KT):
            for hx in range(2):
                hs = slice(hx * 256, (hx + 1) * 256)
                nc.sync.dma_start(x_sb[kt][:, hs], xT[kt * 128:(kt + 1) * 128, hs])
        nc.sync.dma_start(cos_sb[:, 0:IS], cosT[:, 0:IS])
        nc.sync.dma_start(sin_sb[:, 0:IS], sinT[:, 0:IS])
        for kt in range(KT):
            nc.sync.dma_start(w_sb[:, kt, Q01:Q01 + 128], wr[:, kt, Q01:Q01 + 128])
        nc.sync.dma_start(cos_sb[:, IS:N], cosT[:, IS:N])
        nc.sync.dma_start(sin_sb[:, IS:N], sinT[:, IS:N])
        for s in range(1, NSTRIP):
            ss = slice(s * IS, (s + 1) * IS)
            for kt in range(KT):
                nc.sync.dma_start(x_sb[kt][:, ss], xT[kt * 128:(kt + 1) * 128, ss])
        for kt in range(KT):
            nc.sync.dma_start(w_sb[:, kt, V012:V012 + 192], wr[:, kt, V012:V012 + 192])
        nc.sync.dma_start(wp_sb, wp.rearrange("(o p) f -> p o f", p=128))

        nc.vector.memset(bias_sb, EXP_BIAS)
        nc.vector.memset(warm, 0.0)
        # early ACT table load for Exp during the DMA window
        nc.scalar.activation(out=warm_o, in_=warm, func=Exp)
        # ones columns of the (v | ones) PV groups (fp8 1.0 is exact)
        nc.gpsimd.memset(v_sb[:, :, :, :, 64:128], 1.0)

        # ---- projection chain + rope ------------------------------------
        def rope_group(dst, colblk, s, pre_scalar=False):
            """One 128-feature projection chain + rope into dst[:, strip s].

            rotate-half partner via 4 cross-partition bf16 copies; sin sign
            pattern ([-sin;+sin] per 32-row half) baked into sinT."""
            ss = slice(s * IS, (s + 1) * IS)
            pt = wkps.tile([128, IS], F32, name="wk", tag="wk")
            for kt in range(KT):
                nc.tensor.matmul(
                    pt, w_sb[:, kt, colblk:colblk + 128], x_sb[kt][:, ss],
                    start=(kt == 0), stop=(kt == KT - 1),
                )
            qpre = rope_pool.tile([128, IS], BF16, name="qpre", tag="qpre")
            if pre_scalar:
                nc.scalar.copy(out=qpre, in_=pt)
            else:
                nc.vector.tensor_copy(out=qpre, in_=pt)
            qps = rope_pool.tile([128, IS], BF16, name="qps", tag="qps")
            for (a, b) in ((0, 32), (32, 0), (64, 96), (96, 64)):
                nc.vector.tensor_copy(out=qps[a:a + 32, :], in_=qpre[b:b + 32, :])
            tmp1 = rope_pool.tile([128, IS], BF16, name="tmp1", tag="tmp1")
            tmp2 = rope_pool.tile([128, IS], BF16, name="tmp2", tag="tmp2")
            nc.vector.tensor_mul(out=tmp1, in0=qpre, in1=cos_sb[:, ss])
            nc.vector.tensor_mul(out=tmp2, in0=qps, in1=sin_sb[:, ss])
            nc.vector.tensor_add(out=dst[:, ss], in0=tmp1, in1=tmp2)
            if dst is qk2d:
                # matmul needs lhsT/rhs on the same base partition: keep a
                # base-0 copy of k2 for the h2 score matmuls
                nc.vector.tensor_copy(out=k2lo[:, ss], in_=qk2d[64:128, ss])

        def v_block(tb):
            """v for one 128-token block, token-major: x-block stationary,
            v weight columns moving -> [tok, 3*64] -> fp8 scatter into the
            key-block-paired v_sb layout."""
            tbs = slice(tb * 128, (tb + 1) * 128)
            pt = wkps.tile([128, IS], F32, name="wk", tag="wk")
            for kt in range(KT):
                nc.tensor.matmul(
                    pt[:, 0:192], x_sb[kt][:, tbs], w_sb[:, kt, V012:V012 + 192],
                    start=(kt == 0), stop=(kt == KT - 1),
                )
            nc.vector.tensor_copy(
                out=v_sb[:, tb // 2, tb % 2, :, 0:64],
                in_=pt[:, 0:192].rearrange("p (h x) -> p h x", h=3),
            )

        # ---- scores + exp -----------------------------------------------
        def score_group(s, jb):
            ss = slice(s * IS, (s + 1) * IS)
            jbs = slice(jb * 128, (jb + 1) * 128)
            st = stsp.tile([128, 3, IS], F32, name="st", tag="st")
            nc.tensor.matmul(st[:, 0, :], k01[0:64, jbs], q01[0:64, ss],
                             start=True, stop=True)
            nc.tensor.matmul(st[:, 1, :], k01[64:128, jbs], q01[64:128, ss],
                             start=True, stop=True)
            nc.tensor.matmul(st[:, 2, :], k2lo[:, jbs], qk2d[0:64, ss],
                             start=True, stop=True)
            nc.scalar.activation(
                out=e_all[s % 2][:, jb // 2, jb % 2], in_=st,
                func=Exp, bias=bias_sb[:, :],
            )

        # ---- PV (fp8 DoubleRow over key-block pairs) + normalization -----
        pvst = {}

        def pv_mms(ps, h, pv, gs):
            et = e_all[ps % 2]
            for g in gs:
                if PV_FP8:
                    nc.tensor.matmul(
                        pv, v_sb[:, g, :, h, :], et[:, g, :, h, :],
                        start=(g == 0), stop=(g == NJP - 1),
                        perf_mode=DR,
                    )
                else:
                    for m in range(2):
                        nc.tensor.matmul(
                            pv, v_sb[:, g, m, h, :], et[:, g, m, h, :],
                            start=(g == 0 and m == 0),
                            stop=(g == NJP - 1 and m == 1),
                        )

        def pv_start(ps, h):
            pv = wkps.tile([128, IS], F32, name="wk", tag="wk")
            pvst[(ps, h)] = pv
            pv_mms(ps, h, pv, range(0, 4))

        def pv_end(ps, h):
            pv_mms(ps, h, pvst[(ps, h)], range(4, NJP))

        def norm01(ps, use_act=False):
            """h0/h1 denominators packed into one [128,512] reciprocal;
            numerator muls read the PV PSUM directly."""
            ss = slice(ps * IS, (ps + 1) * IS)
            pv0 = pvst.pop((ps, 0))
            pv1 = pvst.pop((ps, 1))
            r01 = nrm.tile([128, IS], F32, name="r01", tag="r01")
            cd = nrm.tile([128, IS], F32, name="cd", tag="cd")
            nc.vector.tensor_copy(out=cd[0:64, :], in_=pv0[64:128, :])
            nc.vector.tensor_copy(out=cd[64:128, :], in_=pv1[64:128, :])
            nc.vector.reciprocal(r01, cd)
            nc.vector.tensor_mul(out=P0[0:64, ss], in0=pv0[0:64, :],
                                 in1=r01[0:64, :])
            nc.vector.tensor_mul(out=P0[64:128, ss], in0=pv1[0:64, :],
                                 in1=r01[64:128, :])

        def norm2(ps, use_act=False):
            ss = slice(ps * IS, (ps + 1) * IS)
            pv2 = pvst.pop((ps, 2))
            r2 = nrm.tile([64, IS], F32, name="r2", tag="r2")
            nc.vector.reciprocal(r2, pv2[64:128, :])
            nc.vector.tensor_mul(out=P1[0:64, ss], in0=pv2[0:64, :], in1=r2)
            nc.vector.tensor_copy(out=P1[64:128, ss], in_=P1[0:64, ss])

        def proj_obs(t, obs):
            ss = slice(t * IS, (t + 1) * IS)
            for ob in obs:
                obsl = slice(ob * 128, (ob + 1) * 128)
                pp = wkps.tile([128, IS], F32, name="wk", tag="wk")
                nc.tensor.matmul(pp, wp_sb[:, 0, obsl], P0[:, ss],
                                 start=True, stop=False)
                nc.tensor.matmul(pp, wp_sb[:, 1, obsl], P1[:, ss],
                                 start=False, stop=True)
                ot = prout.tile([128, IS], BF16, name="ot", tag="ot")
                nc.vector.tensor_copy(out=ot, in_=pp)
                nc.sync.dma_start(outT[obsl, ss], ot)

        # ---- prologue: strip 0's own projections ------------------------
        rope_group(k01, K01, 0, pre_scalar=True)
        rope_group(qk2d, QK2, 0, pre_scalar=True)
        rope_group(q01, Q01, 0, pre_scalar=True)

        # ---- strip 0: scores/exp with phase 1 as filler ------------------
        # k01/qk2 of strip t must land before score group jb=4t.
        s0_fillers = [
            lambda: rope_group(k01, K01, 1),
            lambda: rope_group(qk2d, QK2, 1),
            lambda: rope_group(q01, Q01, 1),
            lambda: v_block(0),
            lambda: rope_group(k01, K01, 2),
            lambda: rope_group(qk2d, QK2, 2),
            lambda: v_block(1),
            lambda: rope_group(q01, Q01, 2),
            lambda: rope_group(k01, K01, 3),
            lambda: rope_group(qk2d, QK2, 3),
            lambda: v_block(2),
            lambda: v_block(3),
            lambda: v_block(4),
            lambda: v_block(5),
            lambda: v_block(6),
            lambda: v_block(7),
        ]
        for jb in range(NJB):
            score_group(0, jb)
            s0_fillers[jb]()

        # ---- strips 1..3 + PV/norm/proj fillers + tail -------------------
        if True:
            strip_fillers = {
                1: [
                    lambda: rope_group(q01, Q01, 3),
                    lambda: v_block(8), lambda: v_block(9),
                    lambda: v_block(10), lambda: v_block(11),
                    lambda: pv_start(0, 0),
                    lambda: v_block(12), lambda: v_block(13),
                    lambda: v_block(14), lambda: v_block(15),
                    lambda: pv_end(0, 0),
                    lambda: pv_start(0, 1), lambda: pv_end(0, 1),
                    lambda: norm01(0),
                    lambda: (pv_start(0, 2), pv_end(0, 2)),
                    lambda: norm2(0),
                ],
                2: [
                    lambda: pv_start(1, 0), lambda: pv_end(1, 0),
                    lambda: pv_start(1, 1), lambda: pv_end(1, 1),
                    lambda: norm01(1),
                    lambda: pv_start(1, 2), lambda: pv_end(1, 2),
                    lambda: norm2(1),
                    lambda: proj_obs(0, [0, 1]),
                    lambda: proj_obs(0, [2, 3]),
                    lambda: proj_obs(0, [4, 5]),
                ],
                3: [
                    lambda: pv_start(2, 0), lambda: pv_end(2, 0),
                    lambda: pv_start(2, 1), lambda: pv_end(2, 1),
                    lambda: norm01(2),
                    lambda: pv_start(2, 2), lambda: pv_end(2, 2),
                    lambda: norm2(2),
                    lambda: proj_obs(1, [0, 1]),
                    lambda: proj_obs(1, [2, 3]),
                    lambda: proj_obs(1, [4, 5]),
                    lambda: proj_obs(2, [0, 1]),
                    lambda: proj_obs(2, [2, 3]),
                    lambda: proj_obs(2, [4, 5]),
                ],
            }
            for s in range(1, NSTRIP):
                fillers = strip_fillers[s]
                fi = 0
                for jb in range(NJB):
                    score_group(s, jb)
                    if fi < len(fillers):
                        fillers[fi]()
                        fi += 1
                while fi < len(fillers):
                    fillers[fi]()
                    fi += 1
                if s == 1:
                    ph1_stack.close()

            # tail: strip 3's PV. h0 steals the score-group PSUM buffer that
            # retires after ACT(14), so it overlaps the last score groups.
            st3 = stsp.tile([128, 3, IS], F32, name="st", tag="st")
            pv30 = st3[:, 0, :]
            pvst[(3, 0)] = pv30
            pv_mms(3, 0, pv30, range(NJP))
            pv_start(3, 1)
            pv_end(3, 1)
            norm01(3, use_act=True)
            pv_start(3, 2)
            pv_end(3, 2)
            norm2(3, use_act=True)
            proj_obs(3, [0, 1])
            proj_obs(3, [2, 3])
            proj_obs(3, [4, 5])

        if dbg is not None:
            nc.sync.dma_start(dbg["q01"], q01)
            nc.sync.dma_start(dbg["k01"], k01)
            nc.sync.dma_start(dbg["qk2d"], qk2d)
            nc.sync.dma_start(dbg["v_sb"], v_sb.rearrange("p a b c d -> p (a b c d)"))
            nc.sync.dma_start(dbg["e0"], e_all[0].rearrange("p a b c d -> p (a b c d)"))
            nc.sync.dma_start(dbg["e1"], e_all[1].rearrange("p a b c d -> p (a b c d)"))
            nc.sync.dma_start(dbg["P0"], P0)
            nc.sync.dma_start(dbg["P1"], P1)


# ---------------------------------------------------------------------------
# Host-side sharding / unsharding
# ---------------------------------------------------------------------------

def _rope_tables():
    inv_freq = 1.0 / (ROPE_THETA ** (np.arange(0, D, 2, dtype=np.float64) / D))
    ang = np.arange(N, dtype=np.float64)[None, :] * inv_freq[:, None]  # [32, N]
    cos64 = np.concatenate([np.cos(ang), np.cos(ang)], axis=0)
    sin64 = np.concatenate([-np.sin(ang), np.sin(ang)], axis=0)
    cosT = np.concatenate([cos64, cos64], axis=0)
    sinT = np.concatenate([sin64, sin64], axis=0)
    return cosT, sinT  # [128, N] float64


def _bf(a):
    import ml_dtypes

    return np.ascontiguousarray(a).astype(ml_dtypes.bfloat16)


def make_core_inputs(x, w_qkv, w_proj):
    """Build the 8 per-core input dicts from full inputs."""
    x = np.asarray(x, dtype=np.float32)
    w_qkv = np.asarray(w_qkv, dtype=np.float32)
    w_proj = np.asarray(w_proj, dtype=np.float32)

    cosT, sinT = _rope_tables()
    cosT, sinT = _bf(cosT), _bf(sinT)
    perm = np.concatenate([np.arange(0, D, 2), np.arange(1, D, 2)])  # de-interleave
    wq, wk, wv = w_qkv[0:C], w_qkv[C: 2 * C], w_qkv[2 * C: 3 * C]
    scale = np.float32(D ** -0.5)
    wpT = np.ascontiguousarray(w_proj.T)  # [in_features, out_channels]

    in_maps = []
    for c in range(NCORES):
        b, g = divmod(c, 4)
        h0, h1, h2 = 3 * g, 3 * g + 1, 3 * g + 2

        def qrow(h):
            return wq[h * D: (h + 1) * D][perm] * scale

        def krow(h):
            return wk[h * D: (h + 1) * D][perm]

        def vrow(h):
            return wv[h * D: (h + 1) * D]

        blocks = [qrow(h0), qrow(h1)]          # Q01
        blocks += [krow(h0), krow(h1)]         # K01
        blocks += [qrow(h2), krow(h2)]         # QK2
        blocks += [vrow(h0), vrow(h1), vrow(h2)]  # V012
        w_feat = np.concatenate(blocks, axis=0).T  # [C, 576]
        wp_rows = np.concatenate(
            [wpT[h0 * D: (h0 + 1) * D], wpT[h1 * D: (h1 + 1) * D],
             0.5 * wpT[h2 * D: (h2 + 1) * D], 0.5 * wpT[h2 * D: (h2 + 1) * D]],
            axis=0,
        )  # [256, C]
        in_maps.append(
            {
                "xT": _bf(x[b].T),
                "w_feat": _bf(w_feat),
                "wp": _bf(wp_rows),
                "cosT": cosT,
                "sinT": sinT,
            }
        )
    return in_maps


def unshard(core_outs, b_proj):
    """Sum the 4 partial projections per batch, transpose, add bias."""
    b_proj = np.asarray(b_proj, dtype=np.float32)
    out = np.empty((B, N, C), dtype=np.float32)
    for b in range(B):
        acc = np.asarray(core_outs[4 * b], dtype=np.float32).copy()
        for g in range(1, 4):
            acc += np.asarray(core_outs[4 * b + g], dtype=np.float32)
        out[b] = acc.T + b_proj
    return out


_NC_CACHE = {}


def get_nc():
    key = (DEBUG_DUMP, PV_FP8)
    if key not in _NC_CACHE:
        _NC_CACHE[key] = build_nc()
    return _NC_CACHE[key]


def run(inputs, trace=False, **spmd_kwargs):
    """Run on hardware; returns (full_output, BassKernelResults)."""
    nc = get_nc()
    in_maps = make_core_inputs(inputs["x"], inputs["w_qkv"], inputs["w_proj"])
    res = bass_utils.run_bass_kernel_spmd(
        nc, in_maps, core_ids=list(range(NCORES)), trace=trace, **spmd_kwargs
    )
    core_outs = [r["outT"] for r in res.results]
    return unshard(core_outs, inputs["b_proj"]), res


def kernel(x, w_qkv, w_proj, b_proj):
    out, _ = run({"x": x, "w_qkv": w_qkv, "w_proj": w_proj, "b_proj": b_proj})
    return out


# revision 13
# speedup vs baseline: 1.1255x; 1.0442x over previous
"""Trainium2 Bass kernel for nn_Attention_7602092114471.

Full multi-head attention block:
  qkv = x @ w_qkv.T ; split q,k,v into 12 heads of d=64
  q = rope(q * d**-0.5) ; k = rope(k)   (lucidrains interleaved RoPE)
  attn = softmax(q @ k.T) ; out = (attn @ v) reassembled, @ w_proj.T + b_proj

Shapes: x [2, 2048, 768], w_qkv [2304, 768], w_proj [768, 768], b_proj [768].

Sharding: 24 (batch, head) pairs -> 8 cores x 3 heads. Core c handles batch
c//4, heads {3g, 3g+1, 3g+2} with g = c%4. Each core computes its heads'
q/k/v projections, attention, and a partial output projection over its
3 heads' feature columns. The host sums the 4 partial projections per batch
(the tensor-parallel all-reduce, done on host during unshard) and adds bias.

v4 design (v1 218us -> v2 209 -> v3 202 -> this). All matmuls bf16.
  * DMA: inputs are pre-packed on the host so every transfer has >=1.5KB
    contiguous runs per partition row (v3's 256-512B runs collapsed
    per-ring bandwidth ~4x and starved both the prologue and the strip-0
    filler chains). x is packed [128, strip, kt, 512]; w is packed
    block-major [128, block, kt, cols]; transfers are split across rings
    (partition quarters / kt pairs) so the prologue set lands in ~4us.
  * One ACT table set for the whole kernel: the exp-only and ln-only
    table entries are masked during compilation so every Exp AND Ln ACT
    resolves to natural_log_exp_and_others -> exactly one ACT_TABLE_LOAD
    (v2 thrashed 13 loads); softmax 1/L runs on DVE reciprocal in-loop
    and on ScalarE Ln/Exp(-x) in the tail where ScalarE is idle.
  * No swap-projection chains: rope's rotate-half partner via 4 cross-
    partition [32,512] bf16 SBUF copies (DVE 4x rate) off a bf16 copy of
    the projection PSUM; sin sign pattern baked into the host table.
  * v produced token-major directly (x-block stationary, v weight columns
    moving) -> [tok, 3*64] PSUM -> one scatter per block. No PE
    transposes; ones columns prefilled by one gpsimd memset.
  * Scores: one [128, 3, 512] PSUM group (3 banks, double-buffered) per
    key block = 3 MMs + ONE 1536-col exp ACT for all 3 heads.
  * Strip 3's PV runs inside the score stream: h0 steals a retiring
    score-group PSUM buffer after jb14, h1/h2 rotate through the work
    banks as late fillers; only the three jb15 matmuls, the tail norms
    and the last projection remain after the final exp.
"""

import os

import numpy as np

import concourse.bass as bass
import concourse.mybir as mybir
import concourse.tile as tile
from concourse import bacc, bass_utils

# Problem constants (hardcoded per contract; kernel.py must be self-contained).
B = 2
N = 2048
C = 768
H = 12
D = 64
ROPE_THETA = 10000.0
NCORES = 8

F32 = mybir.dt.float32
BF16 = mybir.dt.bfloat16

IS = 512                  # strip width (projections and attention i-strips)
NSTRIP = N // IS          # 4
NJB = N // 128            # 16 key blocks
NJP = NJB // 2            # 8 key-block pairs (v_sb/e pairing layout)
KT = C // 128             # 6 contraction tiles for the projections
EXP_BIAS = -2.0           # constant shift inside exp; cancels in normalization

# packed-w block offsets (in the [128, KT*cols] host layout)
WK_OFF = 0                # k0|k1   KT*128
WQK2_OFF = 768            # q2|k2   KT*128
WQ_OFF = 1536             # q0|q1   KT*128
WV_OFF = 2304             # v0|v1|v2  KT*192
WP_COLS = 3456

DEBUG_DUMP = os.environ.get("K_DEBUG_DUMP", "0") == "1"

# ACT table sets whose presence would split Exp and Ln across different
# tables (one reload per switch); masking them makes both resolve to
# natural_log_exp_and_others.
_MASK_ACT_SETS = ("exp_and_others", "natural_log", "exp_and_friends")


def build_nc():
    """Build the per-core Bass module (same NEFF runs SPMD on all 8 cores)."""
    import concourse.bacc as bacc_mod

    orig_tables = bacc_mod.get_activation_tables

    def patched_tables(arch):
        t = orig_tables(arch)
        return {
            name: (set() if name in _MASK_ACT_SETS else funcs)
            for name, funcs in t.items()
        }

    nc = bacc.Bacc(
        "TRN2",
        target_bir_lowering=False,
        debug=False,
        enable_asserts=False,
    )

    xP = nc.dram_tensor("xP", [128, NSTRIP * KT * IS], BF16, kind="ExternalInput").ap()
    wP = nc.dram_tensor("wP", [128, WP_COLS], BF16, kind="ExternalInput").ap()
    wp = nc.dram_tensor("wp", [256, C], BF16, kind="ExternalInput").ap()
    cosT = nc.dram_tensor("cosT", [128, N], BF16, kind="ExternalInput").ap()
    sinT = nc.dram_tensor("sinT", [128, N], BF16, kind="ExternalInput").ap()
    outT = nc.dram_tensor("outT", [C, N], BF16, kind="ExternalOutput").ap()
    dbg = None
    if DEBUG_DUMP:
        dbg = {
            nm: nc.dram_tensor(f"dbg_{nm}", shp, dt, kind="ExternalOutput").ap()
            for nm, shp, dt in [
                ("q01", [128, N], BF16), ("k01", [128, N], BF16),
                ("qk2d", [128, N], BF16),
                ("v_sb", [128, NJP * 2 * 384], BF16),
                ("e0", [128, NJP * 2 * 3 * IS], BF16),
                ("e1", [128, NJP * 2 * 3 * IS], BF16),
                ("P0", [128, N], BF16), ("P1", [128, N], BF16),
            ]
        }

    bacc_mod.get_activation_tables = patched_tables
    try:
        with tile.TileContext(nc) as tc:
            _kernel_body(tc, nc, xP, wP, wp, cosT, sinT, outT, dbg)
        nc.compile()
    finally:
        bacc_mod.get_activation_tables = orig_tables
    return nc


def _kernel_body(tc, nc, xP, wP, wp, cosT, sinT, outT, dbg=None):
    import contextlib

    Exp = mybir.ActivationFunctionType.Exp
    Ln = mybir.ActivationFunctionType.Ln

    ctx = contextlib.ExitStack()
    with ctx:
        persist = ctx.enter_context(tc.tile_pool(name="persist", bufs=1))
        rope_pool = ctx.enter_context(tc.tile_pool(name="rope", bufs=2))
        nrm = ctx.enter_context(tc.tile_pool(name="nrm", bufs=2))
        prout = ctx.enter_context(tc.tile_pool(name="prout", bufs=4))
        attnA = ctx.enter_context(tc.tile_pool(name="attnA", bufs=1))
        # PSUM: 3-bank score groups double-buffered (6) + 2 work banks
        stsp = ctx.enter_context(tc.tile_pool(name="sts", bufs=2, space="PSUM"))
        wkps = ctx.enter_context(tc.tile_pool(name="wkps", bufs=2, space="PSUM"))

        # ---- persistent SBUF tensors -------------------------------------
        q01 = persist.tile([128, N], BF16, name="q01")
        k01 = persist.tile([128, N], BF16, name="k01")
        qk2d = persist.tile([128, N], BF16, name="qk2d")  # q2 rows 0:64 | k2 64:128
        k2lo = persist.tile([64, N], BF16, name="k2lo")   # k2 at base partition 0
        # (v | ones) stationary groups, key-block-paired layout
        v_sb = persist.tile([128, NJP, 2, 3, 128], BF16, name="v_sb")
        P0 = persist.tile([128, N], BF16, name="P0")  # heads h0 | h1
        P1 = persist.tile([128, N], BF16, name="P1")  # h2 duplicated
        wp_sb = persist.tile([128, 2, C], BF16, name="wp_sb")
        bias_sb = persist.tile([128, 1], F32, name="bias_sb")
        warm = persist.tile([128, 64], F32, name="warm")
        warm_o = persist.tile([128, 64], F32, name="warm_o")

        e_all = [
            attnA.tile([128, NJP, 2, 3, IS], BF16, name="e0"),
            attnA.tile([128, NJP, 2, 3, IS], BF16, name="e1"),
        ]

        ph1_stack = contextlib.ExitStack()
        ph1 = ph1_stack.enter_context(tc.tile_pool(name="ph1", bufs=1))
        wk_sb = ph1.tile([128, KT, 128], BF16, name="wk_sb")
        wqk2_sb = ph1.tile([128, KT, 128], BF16, name="wqk2_sb")
        wq_sb = ph1.tile([128, KT, 128], BF16, name="wq_sb")
        wv_sb = ph1.tile([128, KT, 192], BF16, name="wv_sb")
        cos_sb = ph1.tile([128, N], BF16, name="cos_sb")
        sin_sb = ph1.tile([128, N], BF16, name="sin_sb")
        x_sb = ph1.tile([128, NSTRIP, KT, IS], BF16, name="x_sb")

        xPr = xP.rearrange("p (s k c) -> p s k c", s=NSTRIP, k=KT)

        def dma_wblock(dst, off, wd):
            src = wP[:, off:off + KT * wd].rearrange("p (k c) -> p k c", k=KT)
            for q in range(4):
                qs = slice(q * 32, (q + 1) * 32)
                nc.sync.dma_start(dst[qs], src[qs])

        def dma_xstrip(s, split_part=False):
            for j in range(KT // 2):
                js = slice(2 * j, 2 * j + 2)
                if split_part:
                    for hx in range(2):
                        hs = slice(hx * 64, (hx + 1) * 64)
                        nc.sync.dma_start(x_sb[hs, s, js], xPr[hs, s, js])
                else:
                    nc.sync.dma_start(x_sb[:, s, js], xPr[:, s, js])

        def dma_trig(s):
            ss = slice(s * IS, (s + 1) * IS)
            for hx in range(2):
                hs = slice(hx * 64, (hx + 1) * 64)
                nc.sync.dma_start(cos_sb[hs, ss], cosT[hs, ss])
                nc.sync.dma_start(sin_sb[hs, ss], sinT[hs, ss])

        # prologue-critical first: k01/qk2 weight blocks, x strip 0, trig 0
        dma_wblock(wk_sb, WK_OFF, 128)
        dma_wblock(wqk2_sb, WQK2_OFF, 128)
        dma_xstrip(0, split_part=True)
        dma_trig(0)
        dma_wblock(wq_sb, WQ_OFF, 128)
        for s in range(1, NSTRIP):
            dma_xstrip(s)
            dma_trig(s)
        dma_wblock(wv_sb, WV_OFF, 192)
        wpr = wp.rearrange("(o p) f -> p o f", p=128)
        for hx in range(2):
            hs = slice(hx * 64, (hx + 1) * 64)
            nc.sync.dma_start(wp_sb[hs], wpr[hs])

        nc.vector.memset(bias_sb, EXP_BIAS)
        nc.vector.memset(warm, 0.0)
        # early ACT table load during the DMA window
        nc.scalar.activation(out=warm_o, in_=warm, func=Exp)
        # ones columns of the (v | ones) PV groups
        nc.gpsimd.memset(v_sb[:, :, :, :, 64:128], 1.0)

        # ---- projection chain + rope ------------------------------------
        def rope_group(dst, wsrc, s, pre_scalar=False):
            """One 128-feature projection chain + rope into dst[:, strip s].

            rotate-half partner via 4 cross-partition bf16 copies; sin sign
            pattern ([-sin;+sin] per 32-row half) baked into sinT."""
            ss = slice(s * IS, (s + 1) * IS)
            pt = wkps.tile([128, IS], F32, name="wk", tag="wk")
            for kt in range(KT):
                nc.tensor.matmul(
                    pt, wsrc[:, kt, :], x_sb[:, s, kt, :],
                    start=(kt == 0), stop=(kt == KT - 1),
                )
            qpre = rope_pool.tile([128, IS], BF16, name="qpre", tag="qpre")
            if pre_scalar:
                nc.scalar.copy(out=qpre, in_=pt)
            else:
                nc.vector.tensor_copy(out=qpre, in_=pt)
            qps = rope_pool.tile([128, IS], BF16, name="qps", tag="qps")
            for (a, b) in ((0, 32), (32, 0), (64, 96), (96, 64)):
                nc.vector.tensor_copy(out=qps[a:a + 32, :], in_=qpre[b:b + 32, :])
            tmp1 = rope_pool.tile([128, IS], BF16, name="tmp1", tag="tmp1")
            tmp2 = rope_pool.tile([128, IS], BF16, name="tmp2", tag="tmp2")
            nc.vector.tensor_mul(out=tmp1, in0=qpre, in1=cos_sb[:, ss])
            nc.vector.tensor_mul(out=tmp2, in0=qps, in1=sin_sb[:, ss])
            nc.vector.tensor_add(out=dst[:, ss], in0=tmp1, in1=tmp2)
            if dst is qk2d:
                # matmul needs lhsT/rhs on the same base partition: keep a
                # base-0 copy of k2 for the h2 score matmuls
                nc.vector.tensor_copy(out=k2lo[:, ss], in_=qk2d[64:128, ss])

        def v_block(tb):
            """v for one 128-token block, token-major: x-block stationary,
            v weight columns moving -> [tok, 3*64] -> scatter into the
            key-block-paired v_sb layout."""
            s, sb = divmod(tb, 4)
            pt = wkps.tile([128, IS], F32, name="wk", tag="wk")
            for kt in range(KT):
                nc.tensor.matmul(
                    pt[:, 0:192],
                    x_sb[:, s, kt, sb * 128:(sb + 1) * 128],
                    wv_sb[:, kt, :],
                    start=(kt == 0), stop=(kt == KT - 1),
                )
            nc.vector.tensor_copy(
                out=v_sb[:, tb // 2, tb % 2, :, 0:64],
                in_=pt[:, 0:192].rearrange("p (h x) -> p h x", h=3),
            )

        # ---- scores + exp -----------------------------------------------
        def score_group(s, jb):
            ss = slice(s * IS, (s + 1) * IS)
            jbs = slice(jb * 128, (jb + 1) * 128)
            st = stsp.tile([128, 3, IS], F32, name="st", tag="st")
            nc.tensor.matmul(st[:, 0, :], k01[0:64, jbs], q01[0:64, ss],
                             start=True, stop=True)
            nc.tensor.matmul(st[:, 1, :], k01[64:128, jbs], q01[64:128, ss],
                             start=True, stop=True)
            nc.tensor.matmul(st[:, 2, :], k2lo[:, jbs], qk2d[0:64, ss],
                             start=True, stop=True)
            nc.scalar.activation(
                out=e_all[s % 2][:, jb // 2, jb % 2], in_=st,
                func=Exp, bias=bias_sb[:, :],
            )

        # ---- PV + normalization -----------------------------------------
        pvst = {}
        ALLGM = [(g, m) for g in range(NJP) for m in range(2)]

        def pv_mms(ps, h, pv, gms):
            et = e_all[ps % 2]
            for (g, m) in gms:
                nc.tensor.matmul(
                    pv, v_sb[:, g, m, h, :], et[:, g, m, h, :],
                    start=(g == 0 and m == 0),
                    stop=(g == NJP - 1 and m == 1),
                )

        def pv_start(ps, h):
            pv = wkps.tile([128, IS], F32, name="wk", tag="wk")
            pvst[(ps, h)] = pv
            pv_mms(ps, h, pv, ALLGM[:8])

        def pv_end(ps, h):
            pv_mms(ps, h, pvst[(ps, h)], ALLGM[8:])

        def norm01(ps, use_act=False):
            """h0/h1 denominators packed into one [128,512] reciprocal;
            numerator muls read the PV PSUM directly."""
            ss = slice(ps * IS, (ps + 1) * IS)
            pv0 = pvst.pop((ps, 0))
            pv1 = pvst.pop((ps, 1))
            r01 = nrm.tile([128, IS], F32, name="r01", tag="r01")
            if use_act:
                # tail only: ScalarE idle; Ln+Exp share the one table set
                lt = nrm.tile([128, IS], F32, name="lt", tag="lt")
                nc.scalar.activation(out=lt[0:64, :], in_=pv0[64:128, :], func=Ln)
                nc.scalar.activation(out=lt[64:128, :], in_=pv1[64:128, :], func=Ln)
                nc.scalar.activation(out=r01, in_=lt, func=Exp, scale=-1.0)
            else:
                cd = nrm.tile([128, IS], F32, name="cd", tag="cd")
                nc.vector.tensor_copy(out=cd[0:64, :], in_=pv0[64:128, :])
                nc.vector.tensor_copy(out=cd[64:128, :], in_=pv1[64:128, :])
                nc.vector.reciprocal(r01, cd)
            nc.vector.tensor_mul(out=P0[0:64, ss], in0=pv0[0:64, :],
                                 in1=r01[0:64, :])
            nc.vector.tensor_mul(out=P0[64:128, ss], in0=pv1[0:64, :],
                                 in1=r01[64:128, :])

        def norm2(ps, use_act=False):
            ss = slice(ps * IS, (ps + 1) * IS)
            pv2 = pvst.pop((ps, 2))
            r2 = nrm.tile([64, IS], F32, name="r2", tag="r2")
            if use_act:
                t2 = nrm.tile([64, IS], F32, name="t2", tag="t2")
                nc.scalar.activation(out=t2, in_=pv2[64:128, :], func=Ln)
                nc.scalar.activation(out=r2, in_=t2, func=Exp, scale=-1.0)
            else:
                nc.vector.reciprocal(r2, pv2[64:128, :])
            nc.vector.tensor_mul(out=P1[0:64, ss], in0=pv2[0:64, :], in1=r2)
            nc.vector.tensor_copy(out=P1[64:128, ss], in_=P1[0:64, ss])

        def proj_obs(t, obs):
            ss = slice(t * IS, (t + 1) * IS)
            for ob in obs:
                obsl = slice(ob * 128, (ob + 1) * 128)
                pp = wkps.tile([128, IS], F32, name="wk", tag="wk")
                nc.tensor.matmul(pp, wp_sb[:, 0, obsl], P0[:, ss],
                                 start=True, stop=False)
                nc.tensor.matmul(pp, wp_sb[:, 1, obsl], P1[:, ss],
                                 start=False, stop=True)
                ot = prout.tile([128, IS], BF16, name="ot", tag="ot")
                nc.vector.tensor_copy(out=ot, in_=pp)
                nc.sync.dma_start(outT[obsl, ss], ot)

        # ---- prologue: strip 0's own projections ------------------------
        rope_group(k01, wk_sb, 0, pre_scalar=True)
        rope_group(qk2d, wqk2_sb, 0, pre_scalar=True)
        rope_group(q01, wq_sb, 0, pre_scalar=True)

        # ---- strip 0: scores/exp with phase 1 as filler ------------------
        # k01/qk2 of strip t must land before score group jb=4t.
        s0_fillers = [
            lambda: rope_group(k01, wk_sb, 1),
            lambda: rope_group(qk2d, wqk2_sb, 1),
            lambda: rope_group(q01, wq_sb, 1),
            lambda: v_block(0),
            lambda: rope_group(k01, wk_sb, 2),
            lambda: rope_group(qk2d, wqk2_sb, 2),
            lambda: v_block(1),
            lambda: rope_group(q01, wq_sb, 2),
            lambda: rope_group(k01, wk_sb, 3),
            lambda: rope_group(qk2d, wqk2_sb, 3),
            lambda: v_block(2),
            lambda: v_block(3),
            lambda: v_block(4),
            lambda: v_block(5),
            lambda: v_block(6),
            lambda: v_block(7),
        ]
        for jb in range(NJB):
            score_group(0, jb)
            s0_fillers[jb]()

        # ---- strips 1..3 + PV/norm/proj fillers + tail -------------------
        def pv3_main(h):
            pv = wkps.tile([128, IS], F32, name="wk", tag="wk")
            pvst[(3, h)] = pv
            pv_mms(3, h, pv, ALLGM[:15])

        strip_fillers = {
            1: [
                lambda: rope_group(q01, wq_sb, 3),
                lambda: v_block(8), lambda: v_block(9),
                lambda: v_block(10), lambda: v_block(11),
                lambda: pv_start(0, 0),
                lambda: v_block(12), lambda: v_block(13),
                lambda: v_block(14), lambda: v_block(15),
                lambda: pv_end(0, 0),
                lambda: pv_start(0, 1), lambda: pv_end(0, 1),
                lambda: norm01(0),
                lambda: (pv_start(0, 2), pv_end(0, 2)),
                lambda: norm2(0),
            ],
            2: [
                lambda: pv_start(1, 0), lambda: pv_end(1, 0),
                lambda: pv_start(1, 1), lambda: pv_end(1, 1),
                lambda: norm01(1),
                lambda: pv_start(1, 2), lambda: pv_end(1, 2),
                lambda: norm2(1),
                lambda: proj_obs(0, [0, 1]),
                lambda: proj_obs(0, [2, 3]),
                lambda: proj_obs(0, [4, 5]),
            ],
            3: [
                lambda: pv_start(2, 0), lambda: pv_end(2, 0),
                lambda: pv_start(2, 1), lambda: pv_end(2, 1),
                lambda: norm01(2),
                lambda: pv_start(2, 2), lambda: pv_end(2, 2),
                lambda: norm2(2),
                lambda: proj_obs(1, [0, 1]),
                lambda: proj_obs(1, [2, 3]),
                lambda: proj_obs(1, [4, 5]),
                lambda: proj_obs(2, [0, 1]),
                lambda: proj_obs(2, [2, 3]),
                lambda: proj_obs(2, [4, 5]),
                lambda: pv3_main(1),
                lambda: pv3_main(2),
            ],
        }
        for s in range(1, NSTRIP):
            fillers = strip_fillers[s]
            fi = 0
            for jb in range(NJB):
                score_group(s, jb)
                if s == 3 and jb == 14:
                    # steal the score-group PSUM buffer retiring after
                    # ACT(13) so h0's PV overlaps the last score groups
                    st3 = stsp.tile([128, 3, IS], F32, name="st", tag="st")
                    pvst[(3, 0)] = st3[:, 0, :]
                    pv_mms(3, 0, pvst[(3, 0)], ALLGM[:15])
                if fi < len(fillers):
                    fillers[fi]()
                    fi += 1
            while fi < len(fillers):
                fillers[fi]()
                fi += 1
            if s == 1:
                ph1_stack.close()

        # tail: only the jb15 PV matmuls, tail norms, last projection
        pv_mms(3, 0, pvst[(3, 0)], [ALLGM[15]])
        pv_mms(3, 1, pvst[(3, 1)], [ALLGM[15]])
        pv_mms(3, 2, pvst[(3, 2)], [ALLGM[15]])
        norm01(3, use_act=True)
        norm2(3, use_act=True)
        proj_obs(3, [0, 1])
        proj_obs(3, [2, 3])
        proj_obs(3, [4, 5])

        if dbg is not None:
            nc.sync.dma_start(dbg["q01"], q01)
            nc.sync.dma_start(dbg["k01"], k01)
            nc.sync.dma_start(dbg["qk2d"], qk2d)
            nc.sync.dma_start(dbg["v_sb"], v_sb.rearrange("p a b c d -> p (a b c d)"))
            nc.sync.dma_start(dbg["e0"], e_all[0].rearrange("p a b c d -> p (a b c d)"))
            nc.sync.dma_start(dbg["e1"], e_all[1].rearrange("p a b c d -> p (a b c d)"))
            nc.sync.dma_start(dbg["P0"], P0)
            nc.sync.dma_start(dbg["P1"], P1)


# ---------------------------------------------------------------------------
# Host-side sharding / unsharding
# ---------------------------------------------------------------------------

def _rope_tables():
    inv_freq = 1.0 / (ROPE_THETA ** (np.arange(0, D, 2, dtype=np.float64) / D))
    ang = np.arange(N, dtype=np.float64)[None, :] * inv_freq[:, None]  # [32, N]
    cos64 = np.concatenate([np.cos(ang), np.cos(ang)], axis=0)
    sin64 = np.concatenate([-np.sin(ang), np.sin(ang)], axis=0)
    cosT = np.concatenate([cos64, cos64], axis=0)
    sinT = np.concatenate([sin64, sin64], axis=0)
    return cosT, sinT  # [128, N] float64


def _bf(a):
    import ml_dtypes

    return np.ascontiguousarray(a).astype(ml_dtypes.bfloat16)


def make_core_inputs(x, w_qkv, w_proj):
    """Build the 8 per-core input dicts from full inputs."""
    x = np.asarray(x, dtype=np.float32)
    w_qkv = np.asarray(w_qkv, dtype=np.float32)
    w_proj = np.asarray(w_proj, dtype=np.float32)

    cosT, sinT = _rope_tables()
    cosT, sinT = _bf(cosT), _bf(sinT)
    perm = np.concatenate([np.arange(0, D, 2), np.arange(1, D, 2)])  # de-interleave
    wq, wk, wv = w_qkv[0:C], w_qkv[C: 2 * C], w_qkv[2 * C: 3 * C]
    scale = np.float32(D ** -0.5)
    wpT = np.ascontiguousarray(w_proj.T)  # [in_features, out_channels]

    in_maps = []
    for c in range(NCORES):
        b, g = divmod(c, 4)
        h0, h1, h2 = 3 * g, 3 * g + 1, 3 * g + 2

        def qrow(h):
            return wq[h * D: (h + 1) * D][perm] * scale

        def krow(h):
            return wk[h * D: (h + 1) * D][perm]

        def vrow(h):
            return wv[h * D: (h + 1) * D]

        # packed x: [128, strip, kt, 512] so per-(strip, kt-pair) DMA
        # slices have 2KB contiguous runs per partition row
        xT = x[b].T  # [768, 2048]
        xPk = xT.reshape(KT, 128, NSTRIP, IS).transpose(1, 2, 0, 3)
        xPk = xPk.reshape(128, NSTRIP * KT * IS)

        # packed w: block-major [128, (block, kt, cols)]
        def wblock(rows):  # rows [cols_out, 768] -> [128, KT, cols_out]
            wt = rows.T  # [768, cols]
            return wt.reshape(KT, 128, -1).transpose(1, 0, 2)

        wk01 = wblock(np.concatenate([krow(h0), krow(h1)], axis=0))
        wqk2 = wblock(np.concatenate([qrow(h2), krow(h2)], axis=0))
        wq01 = wblock(np.concatenate([qrow(h0), qrow(h1)], axis=0))
        wv012 = wblock(np.concatenate([vrow(h0), vrow(h1), vrow(h2)], axis=0))
        wPk = np.concatenate(
            [wk01.reshape(128, -1), wqk2.reshape(128, -1),
             wq01.reshape(128, -1), wv012.reshape(128, -1)], axis=1
        )  # [128, 3456]

        wp_rows = np.concatenate(
            [wpT[h0 * D: (h0 + 1) * D], wpT[h1 * D: (h1 + 1) * D],
             0.5 * wpT[h2 * D: (h2 + 1) * D], 0.5 * wpT[h2 * D: (h2 + 1) * D]],
            axis=0,
        )  # [256, C]
        in_maps.append(
            {
                "xP": _bf(xPk),
                "wP": _bf(wPk),
                "wp": _bf(wp_rows),
                "cosT": cosT,
                "sinT": sinT,
            }
        )
    return in_maps


def unshard(core_outs, b_proj):
    """Sum the 4 partial projections per batch, transpose, add bias."""
    b_proj = np.asarray(b_proj, dtype=np.float32)
    out = np.empty((B, N, C), dtype=np.float32)
    for b in range(B):
        acc = np.asarray(core_outs[4 * b], dtype=np.float32).copy()
        for g in range(1, 4):
            acc += np.asarray(core_outs[4 * b + g], dtype=np.float32)
        out[b] = acc.T + b_proj
    return out


_NC_CACHE = {}


def get_nc():
    key = (DEBUG_DUMP,)
    if key not in _NC_CACHE:
        _NC_CACHE[key] = build_nc()
    return _NC_CACHE[key]


def run(inputs, trace=False, **spmd_kwargs):
    """Run on hardware; returns (full_output, BassKernelResults)."""
    nc = get_nc()
    in_maps = make_core_inputs(inputs["x"], inputs["w_qkv"], inputs["w_proj"])
    res = bass_utils.run_bass_kernel_spmd(
        nc, in_maps, core_ids=list(range(NCORES)), trace=trace, **spmd_kwargs
    )
    core_outs = [r["outT"] for r in res.results]
    return unshard(core_outs, inputs["b_proj"]), res


def kernel(x, w_qkv, w_proj, b_proj):
    out, _ = run({"x": x, "w_qkv": w_qkv, "w_proj": w_proj, "b_proj": b_proj})
    return out


# revision 15
# speedup vs baseline: 1.2862x; 1.1428x over previous
"""Trainium2 Bass kernel for nn_Attention_7602092114471.

Full multi-head attention block:
  qkv = x @ w_qkv.T ; split q,k,v into 12 heads of d=64
  q = rope(q * d**-0.5) ; k = rope(k)   (lucidrains interleaved RoPE)
  attn = softmax(q @ k.T) ; out = (attn @ v) reassembled, @ w_proj.T + b_proj

Shapes: x [2, 2048, 768], w_qkv [2304, 768], w_proj [768, 768], b_proj [768].

Sharding: 24 (batch, head) pairs -> 8 cores x 3 heads. Core c handles batch
c//4, heads {3g, 3g+1, 3g+2} with g = c%4. Each core computes its heads'
q/k/v projections, attention, and a partial output projection over its
3 heads' feature columns. The host sums the 4 partial projections per batch
(the tensor-parallel all-reduce, done on host during unshard) and adds bias.

v4 design (v1 218us -> v2 209 -> v3 202 -> this). All matmuls bf16.
  * DMA: inputs are pre-packed on the host so every transfer has >=1.5KB
    contiguous runs per partition row (v3's 256-512B runs collapsed
    per-ring bandwidth ~4x and starved both the prologue and the strip-0
    filler chains). x is packed [128, strip, kt, 512]; w is packed
    block-major [128, block, kt, cols]; transfers are split across rings
    (partition quarters / kt pairs) so the prologue set lands in ~4us.
  * One ACT table set for the whole kernel: the exp-only and ln-only
    table entries are masked during compilation so every Exp AND Ln ACT
    resolves to natural_log_exp_and_others -> exactly one ACT_TABLE_LOAD
    (v2 thrashed 13 loads); softmax 1/L runs on DVE reciprocal in-loop
    and on ScalarE Ln/Exp(-x) in the tail where ScalarE is idle.
  * No swap-projection chains: rope's rotate-half partner via 4 cross-
    partition [32,512] bf16 SBUF copies (DVE 4x rate) off a bf16 copy of
    the projection PSUM; sin sign pattern baked into the host table.
  * v produced token-major directly (x-block stationary, v weight columns
    moving) -> [tok, 3*64] PSUM -> one scatter per block. No PE
    transposes; ones columns prefilled by one gpsimd memset.
  * Scores: one [128, 3, 512] PSUM group (3 banks, double-buffered) per
    key block = 3 MMs + ONE 1536-col exp ACT for all 3 heads.
  * Strip 3's PV runs inside the score stream: h0 steals a retiring
    score-group PSUM buffer after jb14, h1/h2 rotate through the work
    banks as late fillers; only the three jb15 matmuls, the tail norms
    and the last projection remain after the final exp.
"""

import os

import numpy as np

import concourse.bass as bass
import concourse.mybir as mybir
import concourse.tile as tile
from concourse import bacc, bass_utils

# Problem constants (hardcoded per contract; kernel.py must be self-contained).
B = 2
N = 2048
C = 768
H = 12
D = 64
ROPE_THETA = 10000.0
NCORES = 8

F32 = mybir.dt.float32
BF16 = mybir.dt.bfloat16

IS = 512                  # strip width (projections and attention i-strips)
NSTRIP = N // IS          # 4
NJB = N // 128            # 16 key blocks
NJP = NJB // 2            # 8 key-block pairs (v_sb/e pairing layout)
KT = C // 128             # 6 contraction tiles for the projections
EXP_BIAS = -2.0           # constant shift inside exp; cancels in normalization

# packed-w block offsets (in the [128, KT*cols] host layout)
WK_OFF = 0                # k0|k1   KT*128
WQK2_OFF = 768            # q2|k2   KT*128
WQ_OFF = 1536             # q0|q1   KT*128
WV_OFF = 2304             # v0|v1|v2  KT*192
WP_COLS = 3456

DEBUG_DUMP = os.environ.get("K_DEBUG_DUMP", "0") == "1"

# ACT table sets whose presence would split Exp and Ln across different
# tables (one reload per switch); masking them makes both resolve to
# natural_log_exp_and_others.
_MASK_ACT_SETS = ("exp_and_others", "natural_log", "exp_and_friends")


def build_nc():
    """Build the per-core Bass module (same NEFF runs SPMD on all 8 cores)."""
    import concourse.bacc as bacc_mod

    orig_tables = bacc_mod.get_activation_tables

    def patched_tables(arch):
        t = orig_tables(arch)
        return {
            name: (set() if name in _MASK_ACT_SETS else funcs)
            for name, funcs in t.items()
        }

    nc = bacc.Bacc(
        "TRN2",
        target_bir_lowering=False,
        debug=False,
        enable_asserts=False,
    )

    xP = nc.dram_tensor("xP", [128, NSTRIP * KT * IS], BF16, kind="ExternalInput").ap()
    wP = nc.dram_tensor("wP", [128, WP_COLS], BF16, kind="ExternalInput").ap()
    wp = nc.dram_tensor("wp", [256, C], BF16, kind="ExternalInput").ap()
    cosT = nc.dram_tensor("cosT", [128, N], BF16, kind="ExternalInput").ap()
    sinT = nc.dram_tensor("sinT", [128, N], BF16, kind="ExternalInput").ap()
    outT = nc.dram_tensor("outT", [C, N], BF16, kind="ExternalOutput").ap()
    dbg = None
    if DEBUG_DUMP:
        dbg = {
            nm: nc.dram_tensor(f"dbg_{nm}", shp, dt, kind="ExternalOutput").ap()
            for nm, shp, dt in [
                ("q01", [128, N], BF16), ("k01", [128, N], BF16),
                ("qk2d", [128, N], BF16),
                ("v_sb", [128, NJP * 2 * 384], BF16),
                ("e0", [128, NJP * 2 * 3 * IS], BF16),
                ("e1", [128, NJP * 2 * 3 * IS], BF16),
                ("P0", [128, N], BF16), ("P1", [128, N], BF16),
            ]
        }

    bacc_mod.get_activation_tables = patched_tables
    try:
        with tile.TileContext(nc) as tc:
            _kernel_body(tc, nc, xP, wP, wp, cosT, sinT, outT, dbg)
        nc.compile()
    finally:
        bacc_mod.get_activation_tables = orig_tables
    return nc


def _kernel_body(tc, nc, xP, wP, wp, cosT, sinT, outT, dbg=None):
    import contextlib

    Exp = mybir.ActivationFunctionType.Exp
    Ln = mybir.ActivationFunctionType.Ln

    ctx = contextlib.ExitStack()
    with ctx:
        persist = ctx.enter_context(tc.tile_pool(name="persist", bufs=1))
        rope_pool = ctx.enter_context(tc.tile_pool(name="rope", bufs=2))
        nrm = ctx.enter_context(tc.tile_pool(name="nrm", bufs=1))
        prout = ctx.enter_context(tc.tile_pool(name="prout", bufs=4))
        attnA = ctx.enter_context(tc.tile_pool(name="attnA", bufs=1))
        # PSUM: 3-bank score groups double-buffered (6) + 2 work banks
        stsp = ctx.enter_context(tc.tile_pool(name="sts", bufs=2, space="PSUM"))
        wkps = ctx.enter_context(tc.tile_pool(name="wkps", bufs=2, space="PSUM"))

        # ---- persistent SBUF tensors -------------------------------------
        q01 = persist.tile([128, N], BF16, name="q01")
        k01 = persist.tile([128, N], BF16, name="k01")
        qk2d = persist.tile([128, N], BF16, name="qk2d")  # q2 rows 0:64 | k2 64:128
        k2lo = persist.tile([64, N], BF16, name="k2lo")   # k2 at base partition 0
        # (v | ones) stationary groups, key-block-paired layout
        v_sb = persist.tile([128, NJP, 2, 3, 128], BF16, name="v_sb")
        P0 = persist.tile([128, N], BF16, name="P0")  # heads h0 | h1
        P1 = persist.tile([128, N], BF16, name="P1")  # h2 duplicated
        wp_sb = persist.tile([128, 2, C], BF16, name="wp_sb")
        bias_sb = persist.tile([128, 1], F32, name="bias_sb")
        warm = persist.tile([128, 64], F32, name="warm")
        warm_o = persist.tile([128, 64], F32, name="warm_o")

        e_all = [
            attnA.tile([128, NJP, 2, 3, IS], BF16, name="e0"),
            attnA.tile([128, NJP, 2, 3, IS], BF16, name="e1"),
        ]

        ph1_stack = contextlib.ExitStack()
        ph1 = ph1_stack.enter_context(tc.tile_pool(name="ph1", bufs=1))
        wk_sb = ph1.tile([128, KT, 128], BF16, name="wk_sb")
        wqk2_sb = ph1.tile([128, KT, 128], BF16, name="wqk2_sb")
        wq_sb = ph1.tile([128, KT, 128], BF16, name="wq_sb")
        wv_sb = ph1.tile([128, KT, 192], BF16, name="wv_sb")
        cos_sb = ph1.tile([128, N], BF16, name="cos_sb")
        sin_sb = ph1.tile([128, N], BF16, name="sin_sb")
        x_sb = ph1.tile([128, NSTRIP, KT, IS], BF16, name="x_sb")

        xPr = xP.rearrange("p (s k c) -> p s k c", s=NSTRIP, k=KT)

        def dma_wblock(dst, off, wd):
            src = wP[:, off:off + KT * wd].rearrange("p (k c) -> p k c", k=KT)
            for q in range(4):
                qs = slice(q * 32, (q + 1) * 32)
                nc.sync.dma_start(dst[qs], src[qs])

        def dma_xstrip(s, split_part=False):
            for j in range(KT // 2):
                js = slice(2 * j, 2 * j + 2)
                if split_part:
                    for hx in range(4):
                        hs = slice(hx * 32, (hx + 1) * 32)
                        nc.sync.dma_start(x_sb[hs, s, js], xPr[hs, s, js])
                else:
                    nc.sync.dma_start(x_sb[:, s, js], xPr[:, s, js])

        def dma_trig(s):
            ss = slice(s * IS, (s + 1) * IS)
            for hx in range(2):
                hs = slice(hx * 64, (hx + 1) * 64)
                nc.sync.dma_start(cos_sb[hs, ss], cosT[hs, ss])
                nc.sync.dma_start(sin_sb[hs, ss], sinT[hs, ss])

        # prologue-critical first: k01 weights + x strip 0 on the earliest
        # rings, then trig/qk2/q01 blocks
        dma_wblock(wk_sb, WK_OFF, 128)
        dma_xstrip(0, split_part=True)
        dma_trig(0)
        dma_wblock(wqk2_sb, WQK2_OFF, 128)
        dma_wblock(wq_sb, WQ_OFF, 128)
        for s in range(1, NSTRIP):
            dma_xstrip(s)
            dma_trig(s)
        dma_wblock(wv_sb, WV_OFF, 192)
        wpr = wp.rearrange("(o p) f -> p o f", p=128)
        for hx in range(2):
            hs = slice(hx * 64, (hx + 1) * 64)
            nc.sync.dma_start(wp_sb[hs], wpr[hs])

        nc.vector.memset(bias_sb, EXP_BIAS)
        nc.vector.memset(warm, 0.0)
        # early ACT table load during the DMA window
        nc.scalar.activation(out=warm_o, in_=warm, func=Exp)
        # ones columns of the (v | ones) PV groups
        nc.gpsimd.memset(v_sb[:, :, :, :, 64:128], 1.0)

        # ---- projection chain + rope ------------------------------------
        def rope_group(dst, wsrc, s, pre_scalar=False):
            """One 128-feature projection chain + rope into dst[:, strip s].

            rotate-half partner via 4 cross-partition bf16 copies; sin sign
            pattern ([-sin;+sin] per 32-row half) baked into sinT."""
            ss = slice(s * IS, (s + 1) * IS)
            pt = wkps.tile([128, IS], F32, name="wk", tag="wk")
            for kt in range(KT):
                nc.tensor.matmul(
                    pt, wsrc[:, kt, :], x_sb[:, s, kt, :],
                    start=(kt == 0), stop=(kt == KT - 1),
                )
            qpre = rope_pool.tile([128, IS], BF16, name="qpre", tag="qpre")
            if pre_scalar:
                nc.scalar.copy(out=qpre, in_=pt)
            else:
                nc.vector.tensor_copy(out=qpre, in_=pt)
            qps = rope_pool.tile([128, IS], BF16, name="qps", tag="qps")
            for (a, b) in ((0, 32), (32, 0), (64, 96), (96, 64)):
                nc.vector.tensor_copy(out=qps[a:a + 32, :], in_=qpre[b:b + 32, :])
            tmp1 = rope_pool.tile([128, IS], BF16, name="tmp1", tag="tmp1")
            tmp2 = rope_pool.tile([128, IS], BF16, name="tmp2", tag="tmp2")
            nc.vector.tensor_mul(out=tmp1, in0=qpre, in1=cos_sb[:, ss])
            nc.vector.tensor_mul(out=tmp2, in0=qps, in1=sin_sb[:, ss])
            nc.vector.tensor_add(out=dst[:, ss], in0=tmp1, in1=tmp2)
            if dst is qk2d:
                # matmul needs lhsT/rhs on the same base partition: keep a
                # base-0 copy of k2 for the h2 score matmuls
                nc.vector.tensor_copy(out=k2lo[:, ss], in_=qk2d[64:128, ss])

        def v_block(tb):
            """v for one 128-token block, token-major: x-block stationary,
            v weight columns moving -> [tok, 3*64] -> scatter into the
            key-block-paired v_sb layout."""
            s, sb = divmod(tb, 4)
            pt = wkps.tile([128, IS], F32, name="wk", tag="wk")
            for kt in range(KT):
                nc.tensor.matmul(
                    pt[:, 0:192],
                    x_sb[:, s, kt, sb * 128:(sb + 1) * 128],
                    wv_sb[:, kt, :],
                    start=(kt == 0), stop=(kt == KT - 1),
                )
            nc.vector.tensor_copy(
                out=v_sb[:, tb // 2, tb % 2, :, 0:64],
                in_=pt[:, 0:192].rearrange("p (h x) -> p h x", h=3),
            )

        # ---- scores + exp -----------------------------------------------
        def score_group(s, jb):
            ss = slice(s * IS, (s + 1) * IS)
            jbs = slice(jb * 128, (jb + 1) * 128)
            st = stsp.tile([128, 3, IS], F32, name="st", tag="st")
            nc.tensor.matmul(st[:, 0, :], k01[0:64, jbs], q01[0:64, ss],
                             start=True, stop=True)
            nc.tensor.matmul(st[:, 1, :], k01[64:128, jbs], q01[64:128, ss],
                             start=True, stop=True)
            nc.tensor.matmul(st[:, 2, :], k2lo[:, jbs], qk2d[0:64, ss],
                             start=True, stop=True)
            nc.scalar.activation(
                out=e_all[s % 2][:, jb // 2, jb % 2], in_=st,
                func=Exp, bias=bias_sb[:, :],
            )

        # ---- PV + normalization -----------------------------------------
        pvst = {}
        ALLGM = [(g, m) for g in range(NJP) for m in range(2)]

        def pv_mms(ps, h, pv, gms):
            et = e_all[ps % 2]
            for (g, m) in gms:
                nc.tensor.matmul(
                    pv, v_sb[:, g, m, h, :], et[:, g, m, h, :],
                    start=(g == 0 and m == 0),
                    stop=(g == NJP - 1 and m == 1),
                )

        def pv_start(ps, h):
            pv = wkps.tile([128, IS], F32, name="wk", tag="wk")
            pvst[(ps, h)] = pv
            pv_mms(ps, h, pv, ALLGM[:8])

        def pv_end(ps, h):
            pv_mms(ps, h, pvst[(ps, h)], ALLGM[8:])

        def norm01(ps, tail=False):
            """h0/h1: numerators+denominators copied out packed (fast PSUM
            bank release), 1/L via ScalarE Ln -> Exp(-x) (the Ln/Exp pair
            lives in the same ACT table set as the score exps)."""
            ss = slice(ps * IS, (ps + 1) * IS)
            pv0 = pvst.pop((ps, 0))
            pv1 = pvst.pop((ps, 1))
            r01 = nrm.tile([128, IS], F32, name="r01", tag="r01")
            lt = nrm.tile([128, IS], F32, name="lt", tag="lt")
            if tail:
                # no bank pressure after the last strip: read PSUM directly
                cn, cd = None, None
                nc.scalar.activation(out=lt[0:64, :], in_=pv0[64:128, :], func=Ln)
                nc.scalar.activation(out=lt[64:128, :], in_=pv1[64:128, :], func=Ln)
            else:
                cn = nrm.tile([128, IS], F32, name="cn", tag="cn")
                cd = nrm.tile([128, IS], F32, name="cd", tag="cd")
                nc.vector.tensor_copy(out=cn[0:64, :], in_=pv0[0:64, :])
                nc.vector.tensor_copy(out=cd[0:64, :], in_=pv0[64:128, :])
                nc.vector.tensor_copy(out=cn[64:128, :], in_=pv1[0:64, :])
                nc.vector.tensor_copy(out=cd[64:128, :], in_=pv1[64:128, :])
                nc.scalar.activation(out=lt, in_=cd, func=Ln)
            nc.scalar.activation(out=r01, in_=lt, func=Exp, scale=-1.0)
            n0 = pv0[0:64, :] if tail else cn[0:64, :]
            n1 = pv1[0:64, :] if tail else cn[64:128, :]
            nc.vector.tensor_mul(out=P0[0:64, ss], in0=n0, in1=r01[0:64, :])
            nc.vector.tensor_mul(out=P0[64:128, ss], in0=n1, in1=r01[64:128, :])

        def norm2(ps, tail=False):
            ss = slice(ps * IS, (ps + 1) * IS)
            pv2 = pvst.pop((ps, 2))
            r2 = nrm.tile([64, IS], F32, name="r2", tag="r2")
            t2 = nrm.tile([64, IS], F32, name="t2", tag="t2")
            if tail:
                cn2 = None
                nc.scalar.activation(out=t2, in_=pv2[64:128, :], func=Ln)
            else:
                cn2 = nrm.tile([64, IS], F32, name="cn2", tag="cn2")
                cd2 = nrm.tile([64, IS], F32, name="cd2", tag="cd2")
                nc.vector.tensor_copy(out=cn2, in_=pv2[0:64, :])
                nc.vector.tensor_copy(out=cd2, in_=pv2[64:128, :])
                nc.scalar.activation(out=t2, in_=cd2, func=Ln)
            nc.scalar.activation(out=r2, in_=t2, func=Exp, scale=-1.0)
            n2 = pv2[0:64, :] if tail else cn2
            nc.vector.tensor_mul(out=P1[0:64, ss], in0=n2, in1=r2)
            nc.vector.tensor_copy(out=P1[64:128, ss], in_=P1[0:64, ss])

        def proj_obs(t, obs, alt_cast=False):
            ss = slice(t * IS, (t + 1) * IS)
            for ob in obs:
                obsl = slice(ob * 128, (ob + 1) * 128)
                pp = wkps.tile([128, IS], F32, name="wk", tag="wk")
                nc.tensor.matmul(pp, wp_sb[:, 0, obsl], P0[:, ss],
                                 start=True, stop=False)
                nc.tensor.matmul(pp, wp_sb[:, 1, obsl], P1[:, ss],
                                 start=False, stop=True)
                ot = prout.tile([128, IS], BF16, name="ot", tag="ot")
                if alt_cast and ob % 2 == 1:
                    nc.scalar.copy(out=ot, in_=pp)
                else:
                    nc.vector.tensor_copy(out=ot, in_=pp)
                nc.sync.dma_start(outT[obsl, ss], ot)

        # ---- prologue: strip 0's own projections ------------------------
        rope_group(k01, wk_sb, 0, pre_scalar=True)
        rope_group(qk2d, wqk2_sb, 0, pre_scalar=True)
        rope_group(q01, wq_sb, 0, pre_scalar=True)

        # ---- strip 0: scores/exp with phase 1 as filler ------------------
        # k01/qk2 of strip t must land before score group jb=4t.
        s0_fillers = [
            lambda: rope_group(k01, wk_sb, 1),
            lambda: rope_group(qk2d, wqk2_sb, 1),
            lambda: rope_group(k01, wk_sb, 2),
            lambda: rope_group(qk2d, wqk2_sb, 2),
            lambda: rope_group(k01, wk_sb, 3),
            lambda: rope_group(qk2d, wqk2_sb, 3),
            lambda: v_block(0),
            lambda: v_block(1),
            lambda: rope_group(q01, wq_sb, 1),
            lambda: v_block(2),
            lambda: v_block(3),
            lambda: rope_group(q01, wq_sb, 2),
            lambda: v_block(4),
            lambda: v_block(5),
            lambda: v_block(6),
            lambda: v_block(7),
        ]
        for jb in range(NJB):
            score_group(0, jb)
            s0_fillers[jb]()

        # ---- strips 1..3 + PV/norm/proj fillers + tail -------------------
        def pv3_main(h):
            pv = wkps.tile([128, IS], F32, name="wk", tag="wk")
            pvst[(3, h)] = pv
            pv_mms(3, h, pv, ALLGM[:15])

        strip_fillers = {
            1: [
                lambda: rope_group(q01, wq_sb, 3),
                lambda: v_block(8), lambda: v_block(9),
                lambda: v_block(10), lambda: v_block(11),
                lambda: pv_start(0, 0),
                lambda: v_block(12), lambda: v_block(13),
                lambda: v_block(14), lambda: v_block(15),
                lambda: pv_end(0, 0),
                lambda: pv_start(0, 1), lambda: pv_end(0, 1),
                lambda: norm01(0),
                lambda: (pv_start(0, 2), pv_end(0, 2)),
                lambda: norm2(0),
            ],
            2: [
                lambda: pv_start(1, 0), lambda: pv_end(1, 0),
                lambda: pv_start(1, 1), lambda: pv_end(1, 1),
                lambda: norm01(1),
                lambda: pv_start(1, 2), lambda: pv_end(1, 2),
                lambda: norm2(1),
                lambda: proj_obs(0, [0, 1]),
                lambda: proj_obs(0, [2, 3]),
                lambda: proj_obs(0, [4, 5]),
            ],
            3: [
                lambda: pv_start(2, 0), lambda: pv_end(2, 0),
                lambda: pv_start(2, 1), lambda: pv_end(2, 1),
                lambda: norm01(2),
                lambda: pv_start(2, 2), lambda: pv_end(2, 2),
                lambda: norm2(2),
                lambda: proj_obs(1, [0, 1]),
                lambda: proj_obs(1, [2, 3]),
                lambda: proj_obs(1, [4, 5]),
                lambda: proj_obs(2, [0, 1]),
                lambda: proj_obs(2, [2, 3]),
                lambda: proj_obs(2, [4, 5]),
                lambda: pv3_main(1),
                lambda: pv3_main(2),
            ],
        }
        for s in range(1, NSTRIP):
            fillers = strip_fillers[s]
            fi = 0
            for jb in range(NJB):
                score_group(s, jb)
                if s == 3 and jb == 14:
                    # steal the score-group PSUM buffer retiring after
                    # ACT(13) so h0's PV overlaps the last score groups
                    st3 = stsp.tile([128, 3, IS], F32, name="st", tag="st")
                    pvst[(3, 0)] = st3[:, 0, :]
                    pv_mms(3, 0, pvst[(3, 0)], ALLGM[:15])
                if fi < len(fillers):
                    fillers[fi]()
                    fi += 1
            while fi < len(fillers):
                fillers[fi]()
                fi += 1
            if s == 1:
                ph1_stack.close()

        # tail: only the jb15 PV matmuls, tail norms, last projection
        pv_mms(3, 0, pvst[(3, 0)], [ALLGM[15]])
        pv_mms(3, 1, pvst[(3, 1)], [ALLGM[15]])
        pv_mms(3, 2, pvst[(3, 2)], [ALLGM[15]])
        norm01(3, tail=True)
        norm2(3, tail=True)
        proj_obs(3, [0, 1], alt_cast=True)
        proj_obs(3, [2, 3], alt_cast=True)
        proj_obs(3, [4, 5], alt_cast=True)

        if dbg is not None:
            nc.sync.dma_start(dbg["q01"], q01)
            nc.sync.dma_start(dbg["k01"], k01)
            nc.sync.dma_start(dbg["qk2d"], qk2d)
            nc.sync.dma_start(dbg["v_sb"], v_sb.rearrange("p a b c d -> p (a b c d)"))
            nc.sync.dma_start(dbg["e0"], e_all[0].rearrange("p a b c d -> p (a b c d)"))
            nc.sync.dma_start(dbg["e1"], e_all[1].rearrange("p a b c d -> p (a b c d)"))
            nc.sync.dma_start(dbg["P0"], P0)
            nc.sync.dma_start(dbg["P1"], P1)


# ---------------------------------------------------------------------------
# Host-side sharding / unsharding
# ---------------------------------------------------------------------------

def _rope_tables():
    inv_freq = 1.0 / (ROPE_THETA ** (np.arange(0, D, 2, dtype=np.float64) / D))
    ang = np.arange(N, dtype=np.float64)[None, :] * inv_freq[:, None]  # [32, N]
    cos64 = np.concatenate([np.cos(ang), np.cos(ang)], axis=0)
    sin64 = np.concatenate([-np.sin(ang), np.sin(ang)], axis=0)
    cosT = np.concatenate([cos64, cos64], axis=0)
    sinT = np.concatenate([sin64, sin64], axis=0)
    return cosT, sinT  # [128, N] float64


def _bf(a):
    import ml_dtypes

    return np.ascontiguousarray(a).astype(ml_dtypes.bfloat16)


def make_core_inputs(x, w_qkv, w_proj):
    """Build the 8 per-core input dicts from full inputs."""
    x = np.asarray(x, dtype=np.float32)
    w_qkv = np.asarray(w_qkv, dtype=np.float32)
    w_proj = np.asarray(w_proj, dtype=np.float32)

    cosT, sinT = _rope_tables()
    cosT, sinT = _bf(cosT), _bf(sinT)
    perm = np.concatenate([np.arange(0, D, 2), np.arange(1, D, 2)])  # de-interleave
    wq, wk, wv = w_qkv[0:C], w_qkv[C: 2 * C], w_qkv[2 * C: 3 * C]
    scale = np.float32(D ** -0.5)
    wpT = np.ascontiguousarray(w_proj.T)  # [in_features, out_channels]

    in_maps = []
    for c in range(NCORES):
        b, g = divmod(c, 4)
        h0, h1, h2 = 3 * g, 3 * g + 1, 3 * g + 2

        def qrow(h):
            return wq[h * D: (h + 1) * D][perm] * scale

        def krow(h):
            return wk[h * D: (h + 1) * D][perm]

        def vrow(h):
            return wv[h * D: (h + 1) * D]

        # packed x: [128, strip, kt, 512] so per-(strip, kt-pair) DMA
        # slices have 2KB contiguous runs per partition row
        xT = x[b].T  # [768, 2048]
        xPk = xT.reshape(KT, 128, NSTRIP, IS).transpose(1, 2, 0, 3)
        xPk = xPk.reshape(128, NSTRIP * KT * IS)

        # packed w: block-major [128, (block, kt, cols)]
        def wblock(rows):  # rows [cols_out, 768] -> [128, KT, cols_out]
            wt = rows.T  # [768, cols]
            return wt.reshape(KT, 128, -1).transpose(1, 0, 2)

        wk01 = wblock(np.concatenate([krow(h0), krow(h1)], axis=0))
        wqk2 = wblock(np.concatenate([qrow(h2), krow(h2)], axis=0))
        wq01 = wblock(np.concatenate([qrow(h0), qrow(h1)], axis=0))
        wv012 = wblock(np.concatenate([vrow(h0), vrow(h1), vrow(h2)], axis=0))
        wPk = np.concatenate(
            [wk01.reshape(128, -1), wqk2.reshape(128, -1),
             wq01.reshape(128, -1), wv012.reshape(128, -1)], axis=1
        )  # [128, 3456]

        wp_rows = np.concatenate(
            [wpT[h0 * D: (h0 + 1) * D], wpT[h1 * D: (h1 + 1) * D],
             0.5 * wpT[h2 * D: (h2 + 1) * D], 0.5 * wpT[h2 * D: (h2 + 1) * D]],
            axis=0,
        )  # [256, C]
        in_maps.append(
            {
                "xP": _bf(xPk),
                "wP": _bf(wPk),
                "wp": _bf(wp_rows),
                "cosT": cosT,
                "sinT": sinT,
            }
        )
    return in_maps


def unshard(core_outs, b_proj):
    """Sum the 4 partial projections per batch, transpose, add bias."""
    b_proj = np.asarray(b_proj, dtype=np.float32)
    out = np.empty((B, N, C), dtype=np.float32)
    for b in range(B):
        acc = np.asarray(core_outs[4 * b], dtype=np.float32).copy()
        for g in range(1, 4):
            acc += np.asarray(core_outs[4 * b + g], dtype=np.float32)
        out[b] = acc.T + b_proj
    return out


_NC_CACHE = {}


def get_nc():
    key = (DEBUG_DUMP,)
    if key not in _NC_CACHE:
        _NC_CACHE[key] = build_nc()
    return _NC_CACHE[key]


def run(inputs, trace=False, **spmd_kwargs):
    """Run on hardware; returns (full_output, BassKernelResults)."""
    nc = get_nc()
    in_maps = make_core_inputs(inputs["x"], inputs["w_qkv"], inputs["w_proj"])
    res = bass_utils.run_bass_kernel_spmd(
        nc, in_maps, core_ids=list(range(NCORES)), trace=trace, **spmd_kwargs
    )
    core_outs = [r["outT"] for r in res.results]
    return unshard(core_outs, inputs["b_proj"]), res


def kernel(x, w_qkv, w_proj, b_proj):
    out, _ = run({"x": x, "w_qkv": w_qkv, "w_proj": w_proj, "b_proj": b_proj})
    return out
